# revision 1
# baseline (speedup 1.0000x reference)
"""3-layer GAT on 8 Trainium2 NeuronCores (Bass/Tile).

Strategy (dst-node graph partition, per sharding hint):
  - Each core owns a contiguous slice of 6250 dst nodes and all edges into them.
  - Per layer: data-parallel projection of the local node slice with an
    extended weight [W | W@al | W@ar] producing packed table rows
    [feat | ex-slot | el] (bf16) plus a local er table; AllGather replicates
    the table; per-edge rows are fetched with dma_gather (two half-tables keep
    indices within int16); attention uses exp without max-subtraction (exact
    softmax is shift-invariant; |e| <= ~2 here); per-128-edge-tile one-hot
    matmuls accumulate both the ex-weighted feature sums and the softmax
    denominators into PSUM per 128-node block; epilogue divides, adds bias,
    applies ELU and PE-transposes into the next layer's lhsT layout.
"""
import sys

import numpy as np
import ml_dtypes

try:
    from concourse import bass, mybir, tile, bacc  # noqa: F401
except ImportError:  # pragma: no cover
    sys.path.insert(0, "/opt/trn_rl_repo")
    from concourse import bass, mybir, tile, bacc  # noqa: F401
from concourse.bass_utils import run_bass_kernel_spmd

bf16 = ml_dtypes.bfloat16
f32 = np.float32

N = 50000
E = 800000
NEG = 0.2
NCORES = 8
NLOC = N // NCORES            # 6250
BLK = 128
NBLK = 49                     # ceil(6250/128)
NLOC_PAD = NBLK * BLK         # 6272
TILE = 128
HALF = 25000                  # nodes below -> table A, rest table B
HALF_ROWS = (HALF // NLOC) * NLOC_PAD   # 25088 padded rows per half-table
CH = 32                       # tiles per gather chunk

import os
DEBUG_PHASE = os.environ.get("KGAT_DEBUG", "")  # "", "proj", "gath", "nomm"

# layer configs: (in_ktiles, H, D, ROW, EXO, ELO, rhsN)
LAYERS = [
    dict(kt=2, H=4, D=32, HD=128, ROW=256, EXO=128, ELO=132, rhsN=132),
    dict(kt=1, H=4, D=32, HD=128, ROW=256, EXO=128, ELO=132, rhsN=132),
    dict(kt=1, H=1, D=64, HD=64, ROW=128, EXO=64, ELO=65, rhsN=65),
]


def _wrap_idx(vals):
    """int16 gather-index layout: element i at [i%16, i//16], replicated to
    all 8 groups of 16 partitions."""
    n = len(vals)
    assert n % 16 == 0
    arr = np.asarray(vals, np.int16).reshape(-1, 16).T  # [16, n//16]
    return np.tile(arr, (8, 1))


def _structure(src, dst):
    """Shared tile schedule + per-core index/one-hot arrays."""
    counts = np.zeros((NCORES, NBLK, 2), np.int64)
    per_core = []
    for k in range(NCORES):
        lo = k * NLOC
        m = (dst >= lo) & (dst < lo + NLOC)
        eidx = np.nonzero(m)[0]
        d_loc = dst[eidx] - lo
        half = (src[eidx] >= HALF).astype(np.int64)
        blk = d_loc // BLK
        order = np.lexsort((d_loc, blk, half))
        eidx, d_loc, half, blk = (a[order] for a in (eidx, d_loc, half, blk))
        per_core.append((eidx, d_loc, half, blk))
        np.add.at(counts[k], (blk, half), 1)
    T = np.maximum(np.ceil(counts / TILE).astype(np.int64).max(axis=0), 1)

    # shared schedule: half-major, block order; tiles per (b, h) = T[b, h]
    tile_block, tile_start, tile_stop, tile_half = [], [], [], []
    for h in range(2):
        for b in range(NBLK):
            for t in range(T[b, h]):
                tile_block.append(b)
                tile_half.append(h)
                tile_start.append(t == 0)
                tile_stop.append(t == T[b, h] - 1)
    S = len(tile_block)
    S_A = int(T[:, 0].sum())

    cores = []
    for k in range(NCORES):
        eidx, d_loc, half, blk = per_core[k]
        src_rows = np.zeros(S * TILE, np.int64)   # half-table row per slot
        oh = np.zeros((128, S * TILE), bf16)
        pos = 0
        for h in range(2):
            for b in range(NBLK):
                sel = np.nonzero((blk == b) & (half == h))[0]
                ns = len(sel)
                sl = slice(pos, pos + ns)
                s_glob = src[eidx[sel]]
                r = (s_glob // NLOC) * NLOC_PAD + s_glob % NLOC
                src_rows[sl] = r - (HALF_ROWS if h else 0)
                slots = pos + np.arange(ns)
                oh[slots % 128, (slots // 128) * 128 +
                   (d_loc[sel] - b * BLK)] = 1.0
                pos += T[b, h] * TILE
        assert src_rows.max() < 32768 and src_rows.min() >= 0
        ohT = np.ascontiguousarray(
            oh.reshape(128, S, TILE).transpose(2, 1, 0)).reshape(
                128, S * TILE)
        cores.append(dict(
            idx_src=_wrap_idx(src_rows),
            oh=oh,
            ohT=ohT,
        ))
    meta = dict(T=T, S=S, S_A=S_A,
                tile_block=tile_block, tile_start=tile_start,
                tile_stop=tile_stop)
    return meta, cores


def _chunks(t0, t1):
    out = []
    t = t0
    while t < t1:
        c = min(CH, t1 - t)
        out.append((t, c))
        t += c
    return out


def _build_program(meta):
    from concourse.masks import make_identity
    dt = mybir.dt
    S, S_A = meta["S"], meta["S_A"]
    tb, tst, tsp = meta["tile_block"], meta["tile_start"], meta["tile_stop"]

    nc = bacc.Bacc("TRN2", target_bir_lowering=False, debug=False,
                   num_devices=NCORES, num_swdge_queues=4)
    xT_in = nc.dram_tensor("xT", [128, 2 * NLOC_PAD], dt.bfloat16,
                           kind="ExternalInput")
    w_in = [nc.dram_tensor(f"W{i+1}", [128, LAYERS[i]["kt"] * (
        LAYERS[i]["HD"] + 2 * LAYERS[i]["H"])], dt.bfloat16,
        kind="ExternalInput") for i in range(3)]
    b_in = [nc.dram_tensor(f"b{i+1}", [128, LAYERS[i]["HD"]], dt.float32,
                           kind="ExternalInput") for i in range(3)]
    isrc_in = nc.dram_tensor("idx_src", [128, S * 8], dt.int16,
                             kind="ExternalInput")
    oh_in = nc.dram_tensor("oh", [128, S * TILE], dt.bfloat16,
                           kind="ExternalInput")
    ohT_in = nc.dram_tensor("ohT", [128, S * TILE], dt.bfloat16,
                            kind="ExternalInput")
    out_ext = nc.dram_tensor("out", [NLOC_PAD, 64], dt.float32,
                             kind="ExternalOutput")

    with tile.TileContext(nc) as tc:
        with (
            tc.tile_pool(name="const", bufs=1) as constp,
            tc.tile_pool(name="acts", bufs=1) as actsp,
            tc.tile_pool(name="stage", bufs=1) as stagep,
            tc.tile_pool(name="ers", bufs=1) as ersp,
            tc.tile_pool(name="stream", bufs=4) as streamp,
            tc.tile_pool(name="epi", bufs=2) as epip,
            tc.tile_pool(name="psA", bufs=2, space="PSUM") as psA,
            tc.tile_pool(name="psB", bufs=2, space="PSUM") as psB,
            tc.tile_pool(name="dram", bufs=1, space="DRAM") as dram,
        ):
            ident = constp.tile([128, 128], dt.bfloat16, tag="ident")
            make_identity(nc, ident[:])
            w_sb, b_sb = [], []
            for i, cfg in enumerate(LAYERS):
                nw = cfg["HD"] + 2 * cfg["H"]
                w = constp.tile([128, cfg["kt"], nw], dt.bfloat16,
                                tag=f"w{i}")
                nc.sync.dma_start(out=w[:], in_=w_in[i][:].rearrange(
                    "p (k c) -> p k c", k=cfg["kt"]))
                w_sb.append(w)
                bb = constp.tile([128, cfg["HD"]], dt.float32, tag=f"b{i}")
                nc.sync.dma_start(out=bb[:], in_=b_in[i][:])
                b_sb.append(bb)

            xT = actsp.tile([128, 2, NLOC_PAD], dt.bfloat16, tag="acts")
            nc.sync.dma_start(out=xT[:], in_=xT_in[:].rearrange(
                "p (k c) -> p k c", k=2))

            hT_prev = xT  # [128, kt, NLOC_PAD] layout; kt collapses via view
            for li, cfg in enumerate(LAYERS):
                H, D, HD = cfg["H"], cfg["D"], cfg["HD"]
                ROW, EXO, ELO, rhsN = (cfg[x] for x in
                                       ("ROW", "EXO", "ELO", "rhsN"))
                kt = cfg["kt"]
                last = li == 2

                tbl_loc = dram.tile([NLOC_PAD, ROW], dt.bfloat16,
                                    tag=f"tl{li}")
                tbl_full = dram.tile([NCORES * NLOC_PAD, ROW], dt.bfloat16,
                                     tag=f"tf{li}")

                # ---- projection: table rows + er table ----
                tbl_sb = stagep.tile([128, NBLK, ROW], dt.bfloat16,
                                     tag="stage")
                er_sb = ersp.tile([128, NBLK, H], dt.bfloat16, tag="ers")
                nc.vector.memset(tbl_sb[:], 0.0)
                for b in range(NBLK):
                    pp = psB.tile([128, HD + 2 * H], dt.float32, tag="proj",
                                  space="PSUM")
                    for k in range(kt):
                        if li == 0:
                            lhsT = hT_prev[:, k, b * BLK:(b + 1) * BLK]
                        else:
                            lhsT = hT_prev[:, b * BLK:(b + 1) * BLK]
                        nc.tensor.matmul(pp[:], lhsT=lhsT, rhs=w_sb[li][:, k, :],
                                         start=(k == 0), stop=(k == kt - 1))
                    nc.vector.tensor_copy(out=tbl_sb[:, b, 0:HD],
                                          in_=pp[:, 0:HD])
                    nc.vector.tensor_copy(out=tbl_sb[:, b, ELO:ELO + H],
                                          in_=pp[:, HD:HD + H])
                    nc.vector.tensor_copy(out=er_sb[:, b, 0:H],
                                          in_=pp[:, HD + H:HD + 2 * H])
                nc.sync.dma_start(
                    out=tbl_loc[:].rearrange("(b p) c -> p b c", p=128),
                    in_=tbl_sb[:])
                nc.gpsimd.collective_compute(
                    "AllGather", mybir.AluOpType.bypass,
                    replica_groups=[list(range(NCORES))],
                    ins=[tbl_loc[:].opt()], outs=[tbl_full[:].opt()])
                # dma_gather ignores AP base offsets on HW; give half B its
                # own tensor via a plain HBM->HBM copy (overlaps half-A work)
                tbl_b = dram.tile([HALF_ROWS, ROW], dt.bfloat16,
                                  tag=f"tb{li}")
                nc.sync.dma_start(out=tbl_b[:],
                                  in_=tbl_full[HALF_ROWS:2 * HALF_ROWS, :])

                if DEBUG_PHASE == "proj":
                    # projection + allgather only; dump a table slice as out
                    dbgb = stagep.tile([128, NBLK, 64], dt.bfloat16,
                                       tag="dbgb", name="dbgb")
                    nc.sync.dma_start(
                        out=dbgb[:],
                        in_=tbl_full[0:NLOC_PAD, 0:64].rearrange(
                            "(b p) c -> p b c", p=128))
                    dbgf = stagep.tile([128, NBLK, 64], dt.float32,
                                       tag="dbgf", name="dbgf")
                    nc.vector.tensor_copy(out=dbgf[:], in_=dbgb[:])
                    nc.sync.dma_start(
                        out=out_ext[:].rearrange("(b p) c -> p b c", p=128),
                        in_=dbgf[:])
                    break

                # ---- edge phase ----
                accA = stagep.tile([128, NBLK, rhsN], dt.float32, tag="stage")
                if last:
                    outacc = stagep.tile([128, NBLK, rhsN], dt.float32,
                                         tag="outacc")

                hT_new = None
                if not last:
                    hT_new = actsp.tile([128, NLOC_PAD], dt.bfloat16,
                                        tag="acts")

                cur = {"psum": None, "b": None, "half": None}
                chunk_no = [0]

                def finish_block(cur=cur, li=li, H=H, D=D, HD=HD, rhsN=rhsN,
                                 accA=accA, hT_new=hT_new, last=last):
                    ps, b, half = cur["psum"], cur["b"], cur["half"]
                    if ps is None:
                        return
                    if half == 0:
                        nc.vector.tensor_copy(out=accA[:, b, :], in_=ps[:])
                        return
                    sm = epip.tile([128, rhsN], mybir.dt.float32, tag="sm")
                    nc.vector.tensor_tensor(out=sm[:], in0=ps[:],
                                            in1=accA[:, b, :],
                                            op=mybir.AluOpType.add)
                    dr = epip.tile([128, H], mybir.dt.float32, tag="dr")
                    nc.vector.tensor_scalar_add(out=dr[:],
                                                in0=sm[:, HD:HD + H],
                                                scalar1=1e-9)
                    nc.vector.reciprocal(out=dr[:], in_=dr[:])
                    q = epip.tile([128, HD], mybir.dt.float32, tag="q")
                    nc.vector.tensor_tensor(
                        out=q[:].rearrange("p (h d) -> p h d", h=H),
                        in0=sm[:, 0:HD].rearrange("p (h d) -> p h d", h=H),
                        in1=dr[:].rearrange("p (h o) -> p h o", h=H)
                            .to_broadcast([128, H, D]),
                        op=mybir.AluOpType.mult)
                    # + bias (host-replicated to all 128 partitions)
                    nc.vector.tensor_tensor(
                        out=q[:], in0=q[:], in1=b_sb[li][:],
                        op=mybir.AluOpType.add)
                    if last:
                        nc.vector.tensor_copy(out=outacc[:, b, 0:HD],
                                              in_=q[:])
                        return
                    # elu: relu(q) + exp(min(q,0)) - 1
                    m = epip.tile([128, HD], mybir.dt.float32, tag="m")
                    nc.vector.tensor_scalar_min(out=m[:], in0=q[:],
                                                scalar1=0.0)
                    nc.scalar.activation(m[:], m[:],
                                         mybir.ActivationFunctionType.Exp)
                    hb = epip.tile([128, HD], mybir.dt.float32, tag="hb")
                    nc.vector.scalar_tensor_tensor(
                        out=hb[:], in0=q[:], scalar=0.0, in1=m[:],
                        op0=mybir.AluOpType.max, op1=mybir.AluOpType.add)
                    hbb = epip.tile([128, HD], mybir.dt.bfloat16, tag="hbb")
                    nc.vector.tensor_scalar_add(out=hbb[:], in0=hb[:],
                                                scalar1=-1.0)
                    tp = psB.tile([128, 128], mybir.dt.bfloat16, tag="tp",
                                  space="PSUM")
                    nc.tensor.transpose(tp[:], hbb[:], ident[:])
                    nc.vector.tensor_copy(
                        out=hT_new[:, b * BLK:(b + 1) * BLK], in_=tp[:])

                for (hf, t0, t1) in ((0, 0, S_A), (1, S_A, S)):
                    tblh = (tbl_full[0:HALF_ROWS, :] if hf == 0
                            else tbl_b[:])
                    for (c0, cn) in _chunks(t0, t1):
                        ni = cn * TILE
                        isb = streamp.tile([128, CH * 8], mybir.dt.int16,
                                           tag="isrc")
                        nc.sync.dma_start(out=isb[:, 0:cn * 8],
                                          in_=isrc_in[:, c0 * 8:c0 * 8 + cn * 8])
                        ohb = streamp.tile([128, CH * TILE], mybir.dt.bfloat16,
                                           tag="oh")
                        nc.sync.dma_start(
                            out=ohb[:, 0:cn * TILE],
                            in_=oh_in[:, c0 * TILE:(c0 + cn) * TILE])
                        ohTb = streamp.tile([128, CH * TILE],
                                            mybir.dt.bfloat16, tag="ohT")
                        nc.sync.dma_start(
                            out=ohTb[:, 0:cn * TILE],
                            in_=ohT_in[:, c0 * TILE:(c0 + cn) * TILE])
                        gath = streamp.tile([128, CH, ROW], mybir.dt.bfloat16,
                                            tag="gath")
                        nc.gpsimd.dma_gather(
                            out_ap=gath[:, 0:cn, :], in_ap=tblh,
                            idxs_ap=isb[:, 0:cn * 8], num_idxs=ni,
                            num_idxs_reg=ni, elem_size=ROW,
                            single_packet=False,
                            queue_num=chunk_no[0] % 4)
                        chunk_no[0] += 1
                        if DEBUG_PHASE == "gathf":
                            continue
                        # er[dst] expansion: per tile OhT.T @ er_block
                        per = psB.tile([128, CH * H], mybir.dt.float32,
                                       tag="er", name="erps", space="PSUM")
                        for t in range(cn):
                            nc.tensor.matmul(
                                per[:, t * H:(t + 1) * H],
                                lhsT=ohTb[:, t * TILE:(t + 1) * TILE],
                                rhs=er_sb[:, tb[c0 + t], 0:H],
                                start=True, stop=True)
                        est = streamp.tile([128, CH, H], mybir.dt.float32,
                                           tag="est")
                        nc.vector.tensor_tensor(
                            out=est[:, 0:cn, :],
                            in0=gath[:, 0:cn, ELO:ELO + H],
                            in1=per[:, 0:cn * H].rearrange(
                                "p (c h) -> p c h", h=H),
                            op=mybir.AluOpType.add)
                        nc.vector.scalar_tensor_tensor(
                            out=est[:, 0:cn, :], in0=est[:, 0:cn, :],
                            scalar=NEG, in1=est[:, 0:cn, :],
                            op0=mybir.AluOpType.mult,
                            op1=mybir.AluOpType.max)
                        nc.scalar.activation(
                            gath[:, 0:cn, EXO:EXO + H], est[:, 0:cn, :],
                            mybir.ActivationFunctionType.Exp)
                        nc.vector.tensor_tensor(
                            out=gath[:, 0:cn, 0:HD].rearrange(
                                "p c (h d) -> p c h d", h=H),
                            in0=gath[:, 0:cn, 0:HD].rearrange(
                                "p c (h d) -> p c h d", h=H),
                            in1=gath[:, 0:cn, EXO:EXO + H]
                                .rearrange("p c (h o) -> p c h o", h=H)
                                .to_broadcast([128, cn, H, D]),
                            op=mybir.AluOpType.mult)
                        if DEBUG_PHASE == "gath":
                            continue
                        for t in range(cn):
                            g = c0 + t
                            if tst[g]:
                                finish_block()
                                cur["psum"] = psA.tile([128, rhsN],
                                                       mybir.dt.float32,
                                                       tag="agg", name="aggp",
                                                       space="PSUM")
                                cur["b"], cur["half"] = tb[g], hf
                            nc.tensor.matmul(
                                cur["psum"][:],
                                lhsT=ohb[:, t * TILE:(t + 1) * TILE],
                                rhs=gath[:, t, 0:rhsN],
                                start=tst[g], stop=tsp[g])
                    finish_block()
                    cur["psum"] = None

                if DEBUG_PHASE.startswith("gath"):
                    dbgf = stagep.tile([128, NBLK, 64], dt.float32,
                                       tag="dbgf", name="dbgf")
                    nc.vector.memset(dbgf[:], 0.0)
                    nc.sync.dma_start(
                        out=out_ext[:].rearrange("(b p) c -> p b c", p=128),
                        in_=dbgf[:])
                    break

                if last:
                    nc.sync.dma_start(
                        out=out_ext[:].rearrange("(b p) c -> p b c", p=128),
                        in_=outacc[:, :, 0:64])
                else:
                    hT_prev = hT_new
    nc.finalize()
    return nc


def kernel(**inputs):
    x = np.asarray(inputs["x"], f32)
    src = np.asarray(inputs["src"]).astype(np.int64)
    dst = np.asarray(inputs["dst"]).astype(np.int64)

    meta, cores = _structure(src, dst)

    # host weight prep: Wext = [W | W@al_h | W@ar_h] per layer
    def wext(W, al, ar):
        W = np.asarray(W, f32)
        al = np.asarray(al, f32)
        ar = np.asarray(ar, f32)
        Hh, Dd = al.shape
        Wl = np.stack([W[:, h * Dd:(h + 1) * Dd] @ al[h] for h in range(Hh)], 1)
        Wr = np.stack([W[:, h * Dd:(h + 1) * Dd] @ ar[h] for h in range(Hh)], 1)
        return np.concatenate([W, Wl, Wr], axis=1)  # [in, HD+2H]

    wx = [wext(inputs["W1"], inputs["al1"], inputs["ar1"]),
          wext(inputs["W2"], inputs["al2"], inputs["ar2"]),
          wext(inputs["W3"], inputs["al3"], inputs["ar3"])]
    w_arrs = []
    for i, cfg in enumerate(LAYERS):
        kt, nw = cfg["kt"], cfg["HD"] + 2 * cfg["H"]
        a = np.zeros((128, kt, nw), bf16)
        for k in range(kt):
            a[:, k, :] = wx[i][k * 128:(k + 1) * 128, :].astype(bf16)
        w_arrs.append(a.reshape(128, kt * nw))
    b_arrs = [np.tile(np.asarray(inputs[f"b{i+1}"], f32).reshape(1, -1),
                      (128, 1)) for i in range(3)]

    nc = _build_program(meta)

    in_maps = []
    for k in range(NCORES):
        lo = k * NLOC
        xT = np.zeros((128, 2, NLOC_PAD), bf16)
        xs = x[lo:lo + NLOC].astype(bf16)     # [6250, 256]
        for kk in range(2):
            xT[:, kk, 0:NLOC] = xs[:, kk * 128:(kk + 1) * 128].T
        in_maps.append({
            "xT": xT.reshape(128, 2 * NLOC_PAD),
            "W1": w_arrs[0], "W2": w_arrs[1], "W3": w_arrs[2],
            "b1": b_arrs[0], "b2": b_arrs[1], "b3": b_arrs[2],
            "idx_src": cores[k]["idx_src"],
            "oh": cores[k]["oh"],
            "ohT": cores[k]["ohT"],
        })

    trace = bool(os.environ.get("KGAT_TRACE"))
    res = run_bass_kernel_spmd(nc, in_maps, core_ids=list(range(NCORES)),
                               trace=trace)
    global LAST_RESULTS
    LAST_RESULTS = res
    out = np.concatenate([res.results[k]["out"][:NLOC]
                          for k in range(NCORES)], axis=0)
    return out.astype(f32)


LAST_RESULTS = None


if __name__ == "__main__":
    import jax
    sys.path.insert(0, "/root/problem")
    import reference as ref
    with jax.default_device(jax.devices("cpu")[0]):
        inp = {k: np.asarray(v) for k, v in ref.setup_inputs().items()}
        expected = np.asarray(ref.reference(**inp))
    got = kernel(**inp)
    err = np.abs(got - expected).max()
    rel = err / np.abs(expected).max()
    print(f"abs err {err:.6f}  rel(absmax) {rel:.6f}")



# revision 4
# speedup vs baseline: 1.0877x; 1.0877x over previous
"""3-layer GAT on 8 Trainium2 NeuronCores (Bass/Tile).

Strategy (dst-node graph partition):
  - Each core owns a contiguous slice of N/8 dst nodes and all edges into
    them. Per layer, nodes are projected data-parallel with an extended
    weight [W | W@al | W@ar]; per-node table rows are packed into 256-BYTE
    elements (the dma_gather sweet spot: ~3 ns/edge vs ~10 ns/edge for
    512B): layers 1-2 store feat as per-(node,head)-scaled int8 plus bf16
    el and bf16 scale; layer 3 stores bf16 feat + el directly. AllGather
    replicates the table; per-edge rows are fetched with dma_gather from
    two half-tables (keeps indices within int16).
  - Attention uses exp without max-subtraction (shift-invariant softmax,
    |e| small); per-128-edge-tile one-hot matmuls accumulate ex-weighted
    feature sums and softmax denominators into per-block PSUM. The int8
    feat is dequantized for free by the ex*scale multiply on Vector.
  - Epilogue (softmax divide, bias, ELU) runs in bulk per ~10-block group,
    and the next layer's projection of each finished group overlaps the
    remaining edge phase; only the AllGather sits between layers.
"""
import os
import sys
from types import SimpleNamespace

import numpy as np
import ml_dtypes

try:
    from concourse import bass, mybir, tile, bacc  # noqa: F401
except ImportError:  # pragma: no cover
    sys.path.insert(0, "/opt/trn_rl_repo")
    from concourse import bass, mybir, tile, bacc  # noqa: F401
from concourse.bass_utils import run_bass_kernel_spmd

bf16 = ml_dtypes.bfloat16
f32 = np.float32

N = int(os.environ.get("KGAT_N", "50000"))
E = int(os.environ.get("KGAT_E", "800000"))
NEG = 0.2
NCORES = 8
NLOC = N // NCORES
BLK = 128
NBLK = (NLOC + BLK - 1) // BLK
NLOC_PAD = NBLK * BLK
TILE = 128
HALF = N // 2
HALF_ROWS = (HALF // NLOC) * NLOC_PAD
CH = 32                      # tiles per gather chunk
TCOL = 128                   # int16 cols per table row (256 bytes)
RND = 1.5 * 2.0 ** 23        # f32 round-to-nearest-int magic constant

NGRP = 5
GROUPS = [(int(g[0]), int(g[-1]) + 1)
          for g in np.array_split(np.arange(NBLK), min(NGRP, NBLK))
          if len(g)]
GMAX = max(b1 - b0 for b0, b1 in GROUPS)

# layer configs; table row (int16 cols): quant: [feat i8 x128 (cols 0:64) |
# el bf16 xH (64:64+H) | scale bf16 xH (68:68+H)]; else bf16 [feat | el].
LAYERS = [
    dict(kt=2, H=4, D=32, HD=128, quant=True),
    dict(kt=1, H=4, D=32, HD=128, quant=True),
    dict(kt=1, H=1, D=64, HD=64, quant=False),
]
ELC = 64     # bf16 col where el starts (byte 128)
SCC = 68     # bf16 col where scale starts (quant layers)


def _wrap_idx(vals):
    n = len(vals)
    assert n % 16 == 0
    arr = np.asarray(vals, np.int16).reshape(-1, 16).T
    return np.tile(arr, (8, 1))


def _structure(src, dst):
    """Shared tile schedule + per-core index/one-hot arrays."""
    counts = np.zeros((NCORES, NBLK, 2), np.int64)
    per_core = []
    for k in range(NCORES):
        lo = k * NLOC
        m = (dst >= lo) & (dst < lo + NLOC)
        eidx = np.nonzero(m)[0]
        d_loc = dst[eidx] - lo
        half = (src[eidx] >= HALF).astype(np.int64)
        blk = d_loc // BLK
        order = np.lexsort((d_loc, blk, half))
        eidx, d_loc, half, blk = (a[order] for a in (eidx, d_loc, half, blk))
        per_core.append((eidx, d_loc, half, blk))
        np.add.at(counts[k], (blk, half), 1)
    T = np.maximum(np.ceil(counts / TILE).astype(np.int64).max(axis=0), 1)

    tile_block, tile_start, tile_stop = [], [], []
    for h in range(2):
        for b in range(NBLK):
            for t in range(T[b, h]):
                tile_block.append(b)
                tile_start.append(t == 0)
                tile_stop.append(t == T[b, h] - 1)
    S = len(tile_block)
    S_A = int(T[:, 0].sum())

    cores = []
    for k in range(NCORES):
        eidx, d_loc, half, blk = per_core[k]
        src_rows = np.zeros(S * TILE, np.int64)
        oh = np.zeros((128, S * TILE), bf16)
        pos = 0
        for h in range(2):
            for b in range(NBLK):
                sel = np.nonzero((blk == b) & (half == h))[0]
                ns = len(sel)
                sl = slice(pos, pos + ns)
                s_glob = src[eidx[sel]]
                r = (s_glob // NLOC) * NLOC_PAD + s_glob % NLOC
                src_rows[sl] = r - (HALF_ROWS if h else 0)
                slots = pos + np.arange(ns)
                oh[slots % 128, (slots // 128) * 128 +
                   (d_loc[sel] - b * BLK)] = 1.0
                pos += T[b, h] * TILE
        assert src_rows.max() < 32768 and src_rows.min() >= 0
        ohT = np.ascontiguousarray(
            oh.reshape(128, S, TILE).transpose(2, 1, 0)).reshape(
                128, S * TILE)
        cores.append(dict(idx_src=_wrap_idx(src_rows), oh=oh, ohT=ohT))
    meta = dict(T=T, S=S, S_A=S_A, tile_block=tile_block,
                tile_start=tile_start, tile_stop=tile_stop)
    return meta, cores


def _chunks(t0, t1):
    out = []
    t = t0
    while t < t1:
        c = min(CH, t1 - t)
        out.append((t, c))
        t += c
    return out


def _build_program(meta):
    from concourse.masks import make_identity
    dt = mybir.dt
    Alu = mybir.AluOpType
    Act = mybir.ActivationFunctionType
    S, S_A = meta["S"], meta["S_A"]
    tb, tst, tsp = meta["tile_block"], meta["tile_start"], meta["tile_stop"]

    nc = bacc.Bacc("TRN2", target_bir_lowering=False, debug=False,
                   num_devices=NCORES, num_swdge_queues=4)
    xT_in = nc.dram_tensor("xT", [128, 2 * NLOC_PAD], dt.bfloat16,
                           kind="ExternalInput")
    w_in = [nc.dram_tensor(f"W{i+1}", [128, LAYERS[i]["kt"] * (
        LAYERS[i]["HD"] + 2 * LAYERS[i]["H"])], dt.bfloat16,
        kind="ExternalInput") for i in range(3)]
    b_in = [nc.dram_tensor(f"b{i+1}", [128, LAYERS[i]["HD"]], dt.float32,
                           kind="ExternalInput") for i in range(3)]
    isrc_in = nc.dram_tensor("idx_src", [128, S * 8], dt.int16,
                             kind="ExternalInput")
    oh_in = nc.dram_tensor("oh", [128, S * TILE], dt.bfloat16,
                           kind="ExternalInput")
    ohT_in = nc.dram_tensor("ohT", [128, S * TILE], dt.bfloat16,
                            kind="ExternalInput")
    out_ext = nc.dram_tensor("out", [NLOC_PAD, 64], dt.float32,
                             kind="ExternalOutput")

    with tile.TileContext(nc) as tc:
        with (
            tc.tile_pool(name="const", bufs=1) as constp,
            tc.tile_pool(name="xblkp", bufs=3) as xblkp,
            tc.tile_pool(name="stage", bufs=1) as stagep,
            tc.tile_pool(name="epi", bufs=1) as epip,
            tc.tile_pool(name="pgath", bufs=5) as pgath,
            tc.tile_pool(name="pwt", bufs=3) as pwt,
            tc.tile_pool(name="poh", bufs=3) as poh,
            tc.tile_pool(name="pohT", bufs=3) as pohT,
            tc.tile_pool(name="pisb", bufs=4) as pisb,
            tc.tile_pool(name="pest", bufs=2) as pest,
            tc.tile_pool(name="psA", bufs=2, space="PSUM") as psA,
            tc.tile_pool(name="psB", bufs=2, space="PSUM") as psB,
            tc.tile_pool(name="dram", bufs=1, space="DRAM") as dram,
        ):
            ident = constp.tile([128, 128], dt.bfloat16, tag="ident")
            make_identity(nc, ident[:])
            w_sb, b_sb = [], []
            for i, cfg in enumerate(LAYERS):
                nw = cfg["HD"] + 2 * cfg["H"]
                w = constp.tile([128, cfg["kt"], nw], dt.bfloat16,
                                tag=f"w{i}")
                nc.sync.dma_start(out=w[:], in_=w_in[i][:].rearrange(
                    "p (k c) -> p k c", k=cfg["kt"]))
                w_sb.append(w)
                bb = constp.tile([128, cfg["HD"]], dt.float32, tag=f"b{i}")
                nc.sync.dma_start(out=bb[:], in_=b_in[i][:])
                b_sb.append(bb)

            tbl_sb = stagep.tile([128, NBLK, TCOL], dt.int16, tag="tblsb")
            nc.vector.memset(tbl_sb[:], 0.0)
            tbl_bf = tbl_sb[:].bitcast(dt.bfloat16)
            tbl_i8 = tbl_sb[:].bitcast(dt.int8)
            er_sb = stagep.tile([128, NBLK, 4], dt.bfloat16, tag="ers")
            accA = stagep.tile([128, NBLK, 132], dt.float32, tag="accA")
            fstage = stagep.tile([128, GMAX, 128], dt.float32, tag="fst")
            elerst = stagep.tile([128, GMAX, 8], dt.float32, tag="elerst")

            tbl_loc = dram.tile([NLOC_PAD, TCOL], dt.int16, tag="tloc")
            tbl_full = dram.tile([NCORES * NLOC_PAD, TCOL], dt.int16,
                                 tag="tfull")
            tbl_b = dram.tile([HALF_ROWS, TCOL], dt.int16, tag="tb")

            def proj_range(li, b0, b1, lhs_of):
                """Project blocks [b0,b1) of layer li into tbl_sb/er_sb."""
                cfg = LAYERS[li]
                H, HD, kt, quant = cfg["H"], cfg["HD"], cfg["kt"], cfg["quant"]
                G = b1 - b0
                for b in range(b0, b1):
                    pp = psB.tile([128, HD + 2 * H], dt.float32, tag="proj",
                                  name="projpp", space="PSUM")
                    for k in range(kt):
                        nc.tensor.matmul(pp[:], lhsT=lhs_of(b, k),
                                         rhs=w_sb[li][:, k, :],
                                         start=(k == 0), stop=(k == kt - 1))
                    if quant:
                        nc.scalar.activation(fstage[:, b - b0, 0:HD],
                                             pp[:, 0:HD], Act.Copy)
                        nc.vector.tensor_copy(out=elerst[:, b - b0, 0:2 * H],
                                              in_=pp[:, HD:HD + 2 * H])
                    else:
                        nc.scalar.activation(tbl_bf[:, b, 0:HD + H],
                                             pp[:, 0:HD + H], Act.Copy)
                        nc.vector.tensor_copy(out=er_sb[:, b, 0:H],
                                              in_=pp[:, HD + H:HD + 2 * H])
                if quant:
                    fv = fstage[:, 0:G, :].rearrange("p g (h d) -> p g h d",
                                                     h=H)
                    mx = epip.tile([128, GMAX, 4], dt.float32, tag="mx",
                                   name="mxt")
                    nc.vector.tensor_reduce(out=mx[:, 0:G, :], in_=fv,
                                            axis=mybir.AxisListType.X,
                                            op=Alu.max,
                                            apply_absolute_value=True)
                    nc.vector.tensor_scalar_max(out=mx[:, 0:G, :],
                                                in0=mx[:, 0:G, :],
                                                scalar1=1e-20)
                    # scale (bf16, stored in table) then rs = 1/scale
                    nc.scalar.activation(tbl_bf[:, b0:b1, SCC:SCC + H],
                                         mx[:, 0:G, :], Act.Copy,
                                         scale=1.0 / 127.0)
                    rs = epip.tile([128, GMAX, 4], dt.float32, tag="rs",
                                   name="rst")
                    nc.vector.reciprocal(out=rs[:, 0:G, :],
                                         in_=tbl_bf[:, b0:b1, SCC:SCC + H])
                    # qint = round(feat * rs): mult, then +RND -RND trick
                    nc.vector.tensor_tensor(
                        out=fv, in0=fv,
                        in1=rs[:, 0:G, :].rearrange("p g (h o) -> p g h o",
                                                    h=H)
                        .to_broadcast([128, G, H, cfg["D"]]),
                        op=Alu.mult)
                    nc.vector.tensor_scalar(out=fv, in0=fv, scalar1=RND,
                                            scalar2=RND, op0=Alu.add,
                                            op1=Alu.subtract)
                    nc.vector.tensor_copy(
                        out=tbl_i8[:, b0:b1, 0:128].rearrange(
                            "p g (h d) -> p g h d", h=H), in_=fv)
                    nc.vector.tensor_copy(out=tbl_bf[:, b0:b1, ELC:ELC + H],
                                          in_=elerst[:, 0:G, 0:H])
                    nc.vector.tensor_copy(out=er_sb[:, b0:b1, 0:H],
                                          in_=elerst[:, 0:G, H:2 * H])

            def store_tables():
                nc.sync.dma_start(
                    out=tbl_loc[:].rearrange("(b p) c -> p b c", p=128),
                    in_=tbl_sb[:])
                nc.gpsimd.collective_compute(
                    "AllGather", mybir.AluOpType.bypass,
                    replica_groups=[list(range(NCORES))],
                    ins=[tbl_loc[:].opt()], outs=[tbl_full[:].opt()])
                # dma_gather ignores AP base offsets on HW; half B gets its
                # own tensor (copy overlaps the next layer's half-A work)
                nc.sync.dma_start(out=tbl_b[:],
                                  in_=tbl_full[HALF_ROWS:2 * HALF_ROWS, :])

            def epilogue_range(li, b0, b1):
                """Softmax-divide + bias (+ELU+transpose or output DMA)."""
                cfg = LAYERS[li]
                H, HD = cfg["H"], cfg["HD"]
                G = b1 - b0
                last = li == 2
                dr = epip.tile([128, GMAX, 4], dt.float32, tag="dr",
                               name="drt")
                nc.vector.tensor_scalar_add(out=dr[:, 0:G, 0:H],
                                            in0=accA[:, b0:b1, HD:HD + H],
                                            scalar1=1e-9)
                nc.vector.reciprocal(out=dr[:, 0:G, 0:H],
                                     in_=dr[:, 0:G, 0:H])
                qt = epip.tile([128, GMAX, 128], dt.float32, tag="qt",
                               name="qtt")
                nc.vector.tensor_tensor(
                    out=qt[:, 0:G, 0:HD].rearrange("p g (h d) -> p g h d",
                                                   h=H),
                    in0=accA[:, b0:b1, 0:HD].rearrange("p g (h d) -> p g h d",
                                                       h=H),
                    in1=dr[:, 0:G, 0:H].rearrange("p g (h o) -> p g h o",
                                                  h=H)
                    .to_broadcast([128, G, H, cfg["D"]]),
                    op=Alu.mult)
                nc.vector.tensor_tensor(
                    out=qt[:, 0:G, 0:HD], in0=qt[:, 0:G, 0:HD],
                    in1=b_sb[li][:].rearrange("p (o c) -> p o c", o=1)
                    .to_broadcast([128, G, HD]),
                    op=Alu.add)
                if last:
                    nc.sync.dma_start(
                        out=out_ext[:].rearrange("(b p) c -> p b c",
                                                 p=128)[:, b0:b1, :],
                        in_=qt[:, 0:G, 0:64])
                    return None
                # elu: relu(q) + exp(min(q,0)) - 1
                m = epip.tile([128, GMAX, 128], dt.float32, tag="m",
                              name="mt")
                nc.vector.tensor_scalar_min(out=m[:, 0:G, 0:HD],
                                            in0=qt[:, 0:G, 0:HD], scalar1=0.0)
                nc.scalar.activation(m[:, 0:G, 0:HD], m[:, 0:G, 0:HD],
                                     Act.Exp)
                hb = epip.tile([128, GMAX, 128], dt.float32, tag="hb",
                               name="hbt")
                nc.vector.scalar_tensor_tensor(
                    out=hb[:, 0:G, 0:HD], in0=qt[:, 0:G, 0:HD], scalar=0.0,
                    in1=m[:, 0:G, 0:HD], op0=Alu.max, op1=Alu.add)
                hbb = epip.tile([128, GMAX, 128], dt.bfloat16, tag="hbb",
                                name="hbbt")
                nc.vector.tensor_scalar_add(out=hbb[:, 0:G, 0:HD],
                                            in0=hb[:, 0:G, 0:HD],
                                            scalar1=-1.0)
                hgrp = epip.tile([128, GMAX * 128], dt.bfloat16, tag="hgrp",
                                 name="hgrpt")
                for b in range(b0, b1):
                    tp = psB.tile([128, 128], dt.bfloat16, tag="tp",
                                  name="tpt", space="PSUM")
                    nc.tensor.transpose(tp[:], hbb[:, b - b0, :], ident[:])
                    nc.scalar.activation(
                        hgrp[:, (b - b0) * 128:(b - b0 + 1) * 128],
                        tp[:], Act.Copy)
                return hgrp

            def edge_phase(li):
                cfg = LAYERS[li]
                H, D, HD, quant = cfg["H"], cfg["D"], cfg["HD"], cfg["quant"]
                rhsN = HD + H
                last = li == 2
                cur = {"psum": None, "b": None, "half": None}
                chunk_no = [0]
                gi = [0]

                def group_done(b):
                    if gi[0] < len(GROUPS) and b == GROUPS[gi[0]][1] - 1:
                        b0, b1 = GROUPS[gi[0]]
                        hgrp = epilogue_range(li, b0, b1)
                        if not last:
                            nli = li + 1
                            def lhs_of(bb, k, hgrp=hgrp, b0=b0):
                                return hgrp[:, (bb - b0) * 128:
                                            (bb - b0 + 1) * 128]
                            proj_range(nli, b0, b1, lhs_of)
                        gi[0] += 1

                def finish_block():
                    ps, b, half = cur["psum"], cur["b"], cur["half"]
                    if ps is None:
                        return
                    if half == 0:
                        nc.scalar.activation(accA[:, b, 0:rhsN], ps[:],
                                             Act.Copy)
                    else:
                        nc.vector.tensor_tensor(out=accA[:, b, 0:rhsN],
                                                in0=ps[:],
                                                in1=accA[:, b, 0:rhsN],
                                                op=Alu.add)
                        group_done(b)
                    cur["psum"] = None

                for (hf, t0, t1) in ((0, 0, S_A), (1, S_A, S)):
                    tblh = tbl_full[0:HALF_ROWS, :] if hf == 0 else tbl_b[:]
                    for (c0, cn) in _chunks(t0, t1):
                        ni = cn * TILE
                        isb = pisb.tile([128, CH * 8], dt.int16, tag="isrc")
                        nc.sync.dma_start(
                            out=isb[:, 0:cn * 8],
                            in_=isrc_in[:, c0 * 8:c0 * 8 + cn * 8])
                        ohb = poh.tile([128, CH * TILE], dt.bfloat16,
                                       tag="oh")
                        nc.sync.dma_start(
                            out=ohb[:, 0:cn * TILE],
                            in_=oh_in[:, c0 * TILE:(c0 + cn) * TILE])
                        ohTb = pohT.tile([128, CH * TILE], dt.bfloat16,
                                         tag="ohT")
                        nc.sync.dma_start(
                            out=ohTb[:, 0:cn * TILE],
                            in_=ohT_in[:, c0 * TILE:(c0 + cn) * TILE])
                        gath = pgath.tile([128, CH, TCOL], dt.int16,
                                          tag="gath")
                        nc.gpsimd.dma_gather(
                            out_ap=gath[:, 0:cn, :], in_ap=tblh,
                            idxs_ap=isb[:, 0:cn * 8], num_idxs=ni,
                            num_idxs_reg=ni, elem_size=TCOL,
                            single_packet=False,
                            queue_num=chunk_no[0] % 4)
                        chunk_no[0] += 1
                        gbf = gath[:, 0:cn, :].bitcast(dt.bfloat16)
                        # er[dst] per slot: per-tile ohT.T @ er_block
                        per = psB.tile([128, CH * 4], dt.float32, tag="er",
                                       name="erps", space="PSUM")
                        for t in range(cn):
                            nc.tensor.matmul(
                                per[:, t * H:(t + 1) * H],
                                lhsT=ohTb[:, t * TILE:(t + 1) * TILE],
                                rhs=er_sb[:, tb[c0 + t], 0:H],
                                start=True, stop=True)
                        est = pest.tile([128, CH, 4], dt.float32, tag="est")
                        nc.vector.tensor_tensor(
                            out=est[:, 0:cn, 0:H],
                            in0=gbf[:, :, ELC:ELC + H],
                            in1=per[:, 0:cn * H].rearrange(
                                "p (c h) -> p c h", h=H),
                            op=Alu.add)
                        nc.vector.scalar_tensor_tensor(
                            out=est[:, 0:cn, 0:H], in0=est[:, 0:cn, 0:H],
                            scalar=NEG, in1=est[:, 0:cn, 0:H],
                            op0=Alu.mult, op1=Alu.max)
                        wt = pwt.tile([128, CH, rhsN], dt.bfloat16, tag="wt")
                        nc.scalar.activation(wt[:, 0:cn, HD:HD + H],
                                             est[:, 0:cn, 0:H], Act.Exp)
                        if quant:
                            # exs = ex * scale (dequant folded in)
                            nc.vector.tensor_tensor(
                                out=est[:, 0:cn, 0:H],
                                in0=wt[:, 0:cn, HD:HD + H],
                                in1=gbf[:, :, SCC:SCC + H],
                                op=Alu.mult)
                            fsrc = gath[:, 0:cn, :].bitcast(dt.int8)[
                                :, :, 0:128].rearrange(
                                    "p c (h d) -> p c h d", h=H)
                            mul_in1 = est[:, 0:cn, 0:H].rearrange(
                                "p c (h o) -> p c h o", h=H).to_broadcast(
                                    [128, cn, H, D])
                        else:
                            fsrc = gbf[:, :, 0:HD].rearrange(
                                "p c (h d) -> p c h d", h=H)
                            mul_in1 = wt[:, 0:cn, HD:HD + H].rearrange(
                                "p c (h o) -> p c h o", h=H).to_broadcast(
                                    [128, cn, H, D])
                        nc.vector.tensor_tensor(
                            out=wt[:, 0:cn, 0:HD].rearrange(
                                "p c (h d) -> p c h d", h=H),
                            in0=fsrc, in1=mul_in1, op=Alu.mult)
                        for t in range(cn):
                            g = c0 + t
                            if tst[g]:
                                finish_block()
                                cur["psum"] = psA.tile(
                                    [128, rhsN], dt.float32, tag="agg",
                                    name="aggp", space="PSUM")
                                cur["b"], cur["half"] = tb[g], hf
                            nc.tensor.matmul(
                                cur["psum"][:],
                                lhsT=ohb[:, t * TILE:(t + 1) * TILE],
                                rhs=wt[:, t, 0:rhsN],
                                start=tst[g], stop=tsp[g])
                    finish_block()

            # ---- layer 1 projection from streamed xT blocks ----
            def xlhs(b, k):
                xb = xblkp.tile([128, 2, 128], dt.bfloat16, tag="xb",
                                name=f"xb{b}")
                if k == 0:
                    nc.sync.dma_start(
                        out=xb[:],
                        in_=xT_in[:].rearrange("p (k c) -> p k c",
                                               k=2)[:, :, b * BLK:(b + 1) * BLK])
                    xlhs.cache[b] = xb
                return xlhs.cache[b][:, k, :]
            xlhs.cache = {}

            for b0, b1 in GROUPS:
                proj_range(0, b0, b1, xlhs)
            store_tables()
            edge_phase(0)      # overlaps layer-2 projection per group
            store_tables()
            edge_phase(1)      # overlaps layer-3 projection per group
            store_tables()
            edge_phase(2)      # writes output per group
    nc.finalize()
    return nc


def kernel(**inputs):
    x = np.asarray(inputs["x"], f32)
    src = np.asarray(inputs["src"]).astype(np.int64)
    dst = np.asarray(inputs["dst"]).astype(np.int64)

    meta, cores = _structure(src, dst)

    def wext(W, al, ar):
        W = np.asarray(W, f32)
        al = np.asarray(al, f32)
        ar = np.asarray(ar, f32)
        Hh, Dd = al.shape
        Wl = np.stack([W[:, h * Dd:(h + 1) * Dd] @ al[h] for h in range(Hh)],
                      1)
        Wr = np.stack([W[:, h * Dd:(h + 1) * Dd] @ ar[h] for h in range(Hh)],
                      1)
        return np.concatenate([W, Wl, Wr], axis=1)

    wx = [wext(inputs["W1"], inputs["al1"], inputs["ar1"]),
          wext(inputs["W2"], inputs["al2"], inputs["ar2"]),
          wext(inputs["W3"], inputs["al3"], inputs["ar3"])]
    w_arrs = []
    for i, cfg in enumerate(LAYERS):
        kt, nw = cfg["kt"], cfg["HD"] + 2 * cfg["H"]
        a = np.zeros((128, kt, nw), bf16)
        for k in range(kt):
            a[:, k, :] = wx[i][k * 128:(k + 1) * 128, :].astype(bf16)
        w_arrs.append(a.reshape(128, kt * nw))
    b_arrs = [np.tile(np.asarray(inputs[f"b{i+1}"], f32).reshape(1, -1),
                      (128, 1)) for i in range(3)]

    nc = _build_program(meta)

    in_maps = []
    for k in range(NCORES):
        lo = k * NLOC
        xT = np.zeros((128, 2, NLOC_PAD), bf16)
        xs = x[lo:lo + NLOC].astype(bf16)
        for kk in range(2):
            xT[:, kk, 0:NLOC] = xs[:, kk * 128:(kk + 1) * 128].T
        in_maps.append({
            "xT": xT.reshape(128, 2 * NLOC_PAD),
            "W1": w_arrs[0], "W2": w_arrs[1], "W3": w_arrs[2],
            "b1": b_arrs[0], "b2": b_arrs[1], "b3": b_arrs[2],
            "idx_src": cores[k]["idx_src"],
            "oh": cores[k]["oh"],
            "ohT": cores[k]["ohT"],
        })

    if os.environ.get("KGAT_SIM"):
        from concourse import bass2jax
        results = bass2jax.run_bass_via_pjrt(nc, in_maps, n_cores=NCORES)
        res = SimpleNamespace(results=results, exec_time_ns=None,
                              instructions_and_trace=None)
    else:
        trace = bool(os.environ.get("KGAT_TRACE"))
        res = run_bass_kernel_spmd(nc, in_maps, core_ids=list(range(NCORES)),
                                   trace=trace)
    global LAST_RESULTS
    LAST_RESULTS = res
    out = np.concatenate([res.results[k]["out"][:NLOC]
                          for k in range(NCORES)], axis=0)
    return out.astype(f32)


LAST_RESULTS = None


# revision 11
# speedup vs baseline: 1.2722x; 1.1697x over previous
"""3-layer GAT on 8 Trainium2 NeuronCores (Bass/Tile).

Strategy (dst-node graph partition):
  - Each core owns a contiguous slice of N/8 dst nodes and all edges into
    them. Per layer, nodes are projected data-parallel with an extended
    weight [W | W@al | W@ar]; per-node table rows are packed into 256-BYTE
    elements (the dma_gather sweet spot: ~3 ns/edge vs ~10 ns/edge for
    512B): layers 1-2 store feat as per-(node,head)-scaled int8 plus bf16
    el and bf16 scale; layer 3 stores bf16 feat + el directly. AllGather
    replicates the table; per-edge rows are fetched with dma_gather from
    two half-tables (keeps indices within int16).
  - Attention uses exp without max-subtraction (shift-invariant softmax,
    |e| small); per-128-edge-tile one-hot matmuls accumulate ex-weighted
    feature sums and softmax denominators into per-block PSUM. The int8
    feat is dequantized for free by the ex*scale multiply on Vector.
  - Epilogue (softmax divide, bias, ELU) runs in bulk per ~10-block group,
    and the next layer's projection of each finished group overlaps the
    remaining edge phase; only the AllGather sits between layers.
"""
import os
import sys
from types import SimpleNamespace

import numpy as np
import ml_dtypes

try:
    from concourse import bass, mybir, tile, bacc  # noqa: F401
except ImportError:  # pragma: no cover
    sys.path.insert(0, "/opt/trn_rl_repo")
    from concourse import bass, mybir, tile, bacc  # noqa: F401
from concourse.bass_utils import run_bass_kernel_spmd

bf16 = ml_dtypes.bfloat16
f32 = np.float32

N = int(os.environ.get("KGAT_N", "50000"))
E = int(os.environ.get("KGAT_E", "800000"))
NEG = 0.2
NCORES = 8
NLOC = N // NCORES
BLK = 128
NBLK = (NLOC + BLK - 1) // BLK
NLOC_PAD = NBLK * BLK
TILE = 128
CH = 32                      # tiles per gather chunk
TCOL = 128                   # int16 cols per table row (256 bytes)
RND = 1.5 * 2.0 ** 23        # f32 round-to-nearest-int magic constant

# src-side split: table part 0 = local blocks [0,PSB), part 1 = [PSB,NBLK);
# each part AllGathers into its own tensor (rows fit int16) and part 0's
# collective fires mid-edge-phase, overlapped with remaining gathers.
PSB = (NBLK + 1) // 2
P_ROWS = [PSB * BLK, (NBLK - PSB) * BLK]
P_SPLIT = PSB * BLK
assert NCORES * max(P_ROWS) < 32768

NGRP_PER_PART = 3
GROUPS = [(int(g[0]), int(g[-1]) + 1)
          for rng in (np.arange(PSB), np.arange(PSB, NBLK))
          for g in np.array_split(rng, min(NGRP_PER_PART, len(rng)))
          if len(g)]
GMAX = max(b1 - b0 for b0, b1 in GROUPS)

# layer configs; table row (int16 cols): quant: [feat i8 x128 (cols 0:64) |
# el bf16 xH (64:64+H) | scale bf16 xH (68:68+H)]; else bf16 [feat | el].
LAYERS = [
    dict(kt=2, H=4, D=32, HD=128, quant=True),
    dict(kt=1, H=4, D=32, HD=128, quant=True),
    dict(kt=1, H=1, D=64, HD=64, quant=False),
]
ELC = 64     # bf16 col where el starts (byte 128)
SCC = 68     # bf16 col where scale starts (quant layers)


def _wrap_idx(vals):
    n = len(vals)
    assert n % 16 == 0
    arr = np.asarray(vals, np.int16).reshape(-1, 16).T
    return np.tile(arr, (8, 1))


def _structure(src, dst):
    """Shared tile schedule + per-core index/one-hot arrays."""
    counts = np.zeros((NCORES, NBLK, 2), np.int64)
    per_core = []
    for k in range(NCORES):
        lo = k * NLOC
        m = (dst >= lo) & (dst < lo + NLOC)
        eidx = np.nonzero(m)[0]
        d_loc = dst[eidx] - lo
        half = ((src[eidx] % NLOC) >= P_SPLIT).astype(np.int64)
        blk = d_loc // BLK
        order = np.lexsort((d_loc, blk, half))
        eidx, d_loc, half, blk = (a[order] for a in (eidx, d_loc, half, blk))
        per_core.append((eidx, d_loc, half, blk))
        np.add.at(counts[k], (blk, half), 1)
    T = np.maximum(np.ceil(counts / TILE).astype(np.int64).max(axis=0), 1)

    tile_block, tile_start, tile_stop = [], [], []
    for h in range(2):
        for b in range(NBLK):
            for t in range(T[b, h]):
                tile_block.append(b)
                tile_start.append(t == 0)
                tile_stop.append(t == T[b, h] - 1)
    S = len(tile_block)
    S_A = int(T[:, 0].sum())

    cores = []
    for k in range(NCORES):
        eidx, d_loc, half, blk = per_core[k]
        src_rows = np.zeros(S * TILE, np.int64)
        oh = np.zeros((128, S * TILE), bf16)
        dcode = np.full((128, S), -1.0, bf16)
        pos = 0
        for h in range(2):
            for b in range(NBLK):
                sel = np.nonzero((blk == b) & (half == h))[0]
                ns = len(sel)
                sl = slice(pos, pos + ns)
                s_glob = src[eidx[sel]]
                loc = s_glob % NLOC
                r = ((s_glob // NLOC) * P_ROWS[h] + loc
                     - (P_SPLIT if h else 0))
                src_rows[sl] = r
                slots = pos + np.arange(ns)
                dc = d_loc[sel] - b * BLK
                oh[slots % 128, (slots // 128) * 128 + dc] = 1.0
                dcode[slots % 128, slots // 128] = dc
                pos += T[b, h] * TILE
        assert src_rows.max() < 32768 and src_rows.min() >= 0
        ohT = np.ascontiguousarray(
            oh.reshape(128, S, TILE).transpose(2, 1, 0)).reshape(
                128, S * TILE)
        cores.append(dict(idx_src=_wrap_idx(src_rows), dcode=dcode, ohT=ohT))
    meta = dict(T=T, S=S, S_A=S_A, tile_block=tile_block,
                tile_start=tile_start, tile_stop=tile_stop)
    return meta, cores


def _chunks(t0, t1):
    out = []
    t = t0
    while t < t1:
        c = min(CH, t1 - t)
        out.append((t, c))
        t += c
    return out


def _build_program(meta):
    from concourse.masks import make_identity
    dt = mybir.dt
    Alu = mybir.AluOpType
    Act = mybir.ActivationFunctionType
    S, S_A = meta["S"], meta["S_A"]
    tb, tst, tsp = meta["tile_block"], meta["tile_start"], meta["tile_stop"]

    nc = bacc.Bacc("TRN2", target_bir_lowering=False, debug=False,
                   num_devices=NCORES, num_swdge_queues=4)
    xT_in = nc.dram_tensor("xT", [128, 2 * NLOC_PAD], dt.bfloat16,
                           kind="ExternalInput")
    w_in = [nc.dram_tensor(f"W{i+1}", [128, LAYERS[i]["kt"] * (
        LAYERS[i]["HD"] + 2 * LAYERS[i]["H"])], dt.bfloat16,
        kind="ExternalInput") for i in range(3)]
    b_in = [nc.dram_tensor(f"b{i+1}", [128, LAYERS[i]["HD"]], dt.float32,
                           kind="ExternalInput") for i in range(3)]
    isrc_in = nc.dram_tensor("idx_src", [128, S * 8], dt.int16,
                             kind="ExternalInput")
    dcode_in = nc.dram_tensor("dcode", [128, S], dt.bfloat16,
                              kind="ExternalInput")
    iota_in = nc.dram_tensor("iota", [128, 128], dt.bfloat16,
                             kind="ExternalInput")
    ohT_in = nc.dram_tensor("ohT", [128, S * TILE], dt.bfloat16,
                            kind="ExternalInput")
    out_ext = nc.dram_tensor("out", [NLOC_PAD, 64], dt.float32,
                             kind="ExternalOutput")

    with tile.TileContext(nc) as tc:
        with (
            tc.tile_pool(name="const", bufs=1) as constp,
            tc.tile_pool(name="xblkp", bufs=3) as xblkp,
            tc.tile_pool(name="stage", bufs=1) as stagep,
            tc.tile_pool(name="epi", bufs=1) as epip,
            tc.tile_pool(name="pgath", bufs=6) as pgath,
            tc.tile_pool(name="pwt", bufs=3) as pwt,
            tc.tile_pool(name="poh", bufs=3) as poh,
            tc.tile_pool(name="pohT", bufs=3) as pohT,
            tc.tile_pool(name="pisb", bufs=4) as pisb,
            tc.tile_pool(name="pest", bufs=2) as pest,
            tc.tile_pool(name="psA", bufs=2, space="PSUM") as psA,
            tc.tile_pool(name="psB", bufs=2, space="PSUM") as psB,
            tc.tile_pool(name="dram", bufs=1, space="DRAM") as dram,
        ):
            ident = constp.tile([128, 128], dt.bfloat16, tag="ident")
            make_identity(nc, ident[:])
            w_sb, b_sb = [], []
            for i, cfg in enumerate(LAYERS):
                nw = cfg["HD"] + 2 * cfg["H"]
                w = constp.tile([128, cfg["kt"], nw], dt.bfloat16,
                                tag=f"w{i}")
                nc.sync.dma_start(out=w[:], in_=w_in[i][:].rearrange(
                    "p (k c) -> p k c", k=cfg["kt"]))
                w_sb.append(w)
                bb = constp.tile([128, cfg["HD"]], dt.float32, tag=f"b{i}")
                nc.sync.dma_start(out=bb[:], in_=b_in[i][:])
                b_sb.append(bb)

            tbl_sb = stagep.tile([128, NBLK, TCOL], dt.int16, tag="tblsb")
            nc.vector.memset(tbl_sb[:], 0.0)
            tbl_bf = tbl_sb[:].bitcast(dt.bfloat16)
            tbl_i8 = tbl_sb[:].bitcast(dt.int8)
            er_sb = stagep.tile([128, NBLK, 4], dt.bfloat16, tag="ers")
            accA = stagep.tile([128, NBLK, 132], dt.float32, tag="accA")
            fstage0 = stagep.tile([128, GMAX, 128], dt.float32, tag="fst0")
            elerst0 = stagep.tile([128, GMAX, 8], dt.float32, tag="elerst0")
            fstage1 = stagep.tile([128, GMAX, 128], dt.float32, tag="fst1")
            elerst1 = stagep.tile([128, GMAX, 8], dt.float32, tag="elerst1")
            PBUFS = [(fstage0, elerst0), (fstage1, elerst1)]

            t_loc = [dram.tile([P_ROWS[0], TCOL], dt.int16, tag="tloc0",
                               name="tloc0"),
                     dram.tile([P_ROWS[1], TCOL], dt.int16, tag="tloc1",
                               name="tloc1")]
            t_full = [dram.tile([NCORES * P_ROWS[0], TCOL], dt.int16,
                                tag="tfull0", name="tfull0"),
                      dram.tile([NCORES * P_ROWS[1], TCOL], dt.int16,
                                tag="tfull1", name="tfull1")]
            dcd = constp.tile([128, S], dt.bfloat16, tag="dcd")
            nc.sync.dma_start(out=dcd[:], in_=dcode_in[:])
            iot = constp.tile([128, 128], dt.bfloat16, tag="iot")
            nc.sync.dma_start(out=iot[:], in_=iota_in[:])

            def proj_stage(li, b0, b1, lhs_of, pp_buf):
                """Projection matmuls + staging for blocks [b0,b1)."""
                cfg = LAYERS[li]
                H, HD, kt, quant = cfg["H"], cfg["HD"], cfg["kt"], cfg["quant"]
                fst, elst = pp_buf
                for b in range(b0, b1):
                    pp = psB.tile([128, HD + 2 * H], dt.float32, tag="proj",
                                  name="projpp", space="PSUM")
                    for k in range(kt):
                        nc.tensor.matmul(pp[:], lhsT=lhs_of(b, k),
                                         rhs=w_sb[li][:, k, :],
                                         start=(k == 0), stop=(k == kt - 1))
                    if quant:
                        nc.scalar.activation(fst[:, b - b0, 0:HD],
                                             pp[:, 0:HD], Act.Copy)
                        nc.vector.tensor_copy(out=elst[:, b - b0, 0:2 * H],
                                              in_=pp[:, HD:HD + 2 * H])
                    else:
                        nc.scalar.activation(tbl_bf[:, b, 0:HD + H],
                                             pp[:, 0:HD + H], Act.Copy)
                        nc.vector.tensor_copy(out=er_sb[:, b, 0:H],
                                              in_=pp[:, HD + H:HD + 2 * H])

            def quant_flush(li, b0, b1, pp_buf):
                """Quantize staged blocks into the int8 table (quant layers)."""
                cfg = LAYERS[li]
                H = cfg["H"]
                G = b1 - b0
                fst, elst = pp_buf
                fv = fst[:, 0:G, :].rearrange("p g (h d) -> p g h d", h=H)
                mx = epip.tile([128, GMAX, 4], dt.float32, tag="mx",
                               name="mxt")
                nc.vector.tensor_reduce(out=mx[:, 0:G, :], in_=fv,
                                        axis=mybir.AxisListType.X,
                                        op=Alu.max,
                                        apply_absolute_value=True)
                nc.vector.tensor_scalar_max(out=mx[:, 0:G, :],
                                            in0=mx[:, 0:G, :],
                                            scalar1=1e-20)
                # scale (bf16, stored in table) then rs = 1/scale
                nc.scalar.activation(tbl_bf[:, b0:b1, SCC:SCC + H],
                                     mx[:, 0:G, :], Act.Copy,
                                     scale=1.0 / 127.0)
                rs = epip.tile([128, GMAX, 4], dt.float32, tag="rs",
                               name="rst")
                nc.vector.reciprocal(out=rs[:, 0:G, :],
                                     in_=tbl_bf[:, b0:b1, SCC:SCC + H])
                # qint = round(feat * rs): mult, then +RND -RND trick
                nc.vector.tensor_tensor(
                    out=fv, in0=fv,
                    in1=rs[:, 0:G, :].rearrange("p g (h o) -> p g h o", h=H)
                    .to_broadcast([128, G, H, cfg["D"]]),
                    op=Alu.mult)
                nc.vector.tensor_scalar_add(out=fv, in0=fv, scalar1=RND)
                nc.vector.tensor_scalar_add(out=fv, in0=fv, scalar1=-RND)
                nc.vector.tensor_copy(
                    out=tbl_i8[:, b0:b1, 0:128].rearrange(
                        "p g (h d) -> p g h d", h=H), in_=fv)
                nc.vector.tensor_copy(out=tbl_bf[:, b0:b1, ELC:ELC + H],
                                      in_=elst[:, 0:G, 0:H])
                nc.vector.tensor_copy(out=er_sb[:, b0:b1, 0:H],
                                      in_=elst[:, 0:G, H:2 * H])

            def store_part(part):
                b0, b1 = (0, PSB) if part == 0 else (PSB, NBLK)
                nc.sync.dma_start(
                    out=t_loc[part][:].rearrange("(b p) c -> p b c", p=128),
                    in_=tbl_sb[:, b0:b1, :])
                nc.gpsimd.collective_compute(
                    "AllGather", mybir.AluOpType.bypass,
                    replica_groups=[list(range(NCORES))],
                    ins=[t_loc[part][:].opt()],
                    outs=[t_full[part][:].opt()])

            def epilogue_range(li, b0, b1):
                """Softmax-divide + bias (+ELU+transpose or output DMA)."""
                cfg = LAYERS[li]
                H, HD = cfg["H"], cfg["HD"]
                G = b1 - b0
                last = li == 2
                dr = epip.tile([128, GMAX, 4], dt.float32, tag="dr",
                               name="drt")
                nc.vector.tensor_scalar_add(out=dr[:, 0:G, 0:H],
                                            in0=accA[:, b0:b1, HD:HD + H],
                                            scalar1=1e-9)
                nc.vector.reciprocal(out=dr[:, 0:G, 0:H],
                                     in_=dr[:, 0:G, 0:H])
                qt = epip.tile([128, GMAX, 128], dt.float32, tag="qt",
                               name="qtt")
                nc.vector.tensor_tensor(
                    out=qt[:, 0:G, 0:HD].rearrange("p g (h d) -> p g h d",
                                                   h=H),
                    in0=accA[:, b0:b1, 0:HD].rearrange("p g (h d) -> p g h d",
                                                       h=H),
                    in1=dr[:, 0:G, 0:H].rearrange("p g (h o) -> p g h o",
                                                  h=H)
                    .to_broadcast([128, G, H, cfg["D"]]),
                    op=Alu.mult)
                nc.vector.tensor_tensor(
                    out=qt[:, 0:G, 0:HD], in0=qt[:, 0:G, 0:HD],
                    in1=b_sb[li][:].rearrange("p (o c) -> p o c", o=1)
                    .to_broadcast([128, G, HD]),
                    op=Alu.add)
                if last:
                    nc.sync.dma_start(
                        out=out_ext[:].rearrange("(b p) c -> p b c",
                                                 p=128)[:, b0:b1, :],
                        in_=qt[:, 0:G, 0:64])
                    return None
                # elu: relu(q) + exp(min(q,0)) - 1
                m = epip.tile([128, GMAX, 128], dt.float32, tag="m",
                              name="mt")
                nc.vector.tensor_scalar_min(out=m[:, 0:G, 0:HD],
                                            in0=qt[:, 0:G, 0:HD], scalar1=0.0)
                nc.scalar.activation(m[:, 0:G, 0:HD], m[:, 0:G, 0:HD],
                                     Act.Exp)
                hb = epip.tile([128, GMAX, 128], dt.float32, tag="hb",
                               name="hbt")
                nc.vector.scalar_tensor_tensor(
                    out=hb[:, 0:G, 0:HD], in0=qt[:, 0:G, 0:HD], scalar=0.0,
                    in1=m[:, 0:G, 0:HD], op0=Alu.max, op1=Alu.add)
                hbb = epip.tile([128, GMAX, 128], dt.bfloat16, tag="hbb",
                                name="hbbt")
                nc.vector.tensor_scalar_add(out=hbb[:, 0:G, 0:HD],
                                            in0=hb[:, 0:G, 0:HD],
                                            scalar1=-1.0)
                hgrp = epip.tile([128, GMAX * 128], dt.bfloat16, tag="hgrp",
                                 name="hgrpt")
                for b in range(b0, b1):
                    tp = psB.tile([128, 128], dt.bfloat16, tag="tp",
                                  name="tpt", space="PSUM")
                    nc.tensor.transpose(tp[:], hbb[:, b - b0, :], ident[:])
                    nc.scalar.activation(
                        hgrp[:, (b - b0) * 128:(b - b0 + 1) * 128],
                        tp[:], Act.Copy)
                return hgrp

            def edge_phase(li):
                cfg = LAYERS[li]
                H, D, HD, quant = cfg["H"], cfg["D"], cfg["HD"], cfg["quant"]
                rhsN = HD + H
                last = li == 2
                cur = {"psum": None, "b": None, "half": None}
                chunk_no = [0]
                gi = [0]

                pending = []

                def flush_pending():
                    if not pending:
                        return
                    p = pending.pop()
                    if LAYERS[p[0]]["quant"]:
                        quant_flush(*p[:3], p[3])
                    if p[2] == PSB:
                        store_part(0)
                    elif p[2] == NBLK:
                        store_part(1)

                def group_done(b):
                    if gi[0] < len(GROUPS) and b == GROUPS[gi[0]][1] - 1:
                        b0, b1 = GROUPS[gi[0]]
                        flush_pending()
                        hgrp = epilogue_range(li, b0, b1)
                        if not last:
                            nli = li + 1
                            def lhs_of(bb, k, hgrp=hgrp, b0=b0):
                                return hgrp[:, (bb - b0) * 128:
                                            (bb - b0 + 1) * 128]
                            proj_stage(nli, b0, b1, lhs_of,
                                       PBUFS[gi[0] % 2])
                            pending.append((nli, b0, b1, PBUFS[gi[0] % 2]))
                        gi[0] += 1

                def finish_block():
                    ps, b, half = cur["psum"], cur["b"], cur["half"]
                    if ps is None:
                        return
                    if half == 0:
                        nc.scalar.activation(accA[:, b, 0:rhsN], ps[:],
                                             Act.Copy)
                    else:
                        nc.vector.tensor_tensor(out=accA[:, b, 0:rhsN],
                                                in0=ps[:],
                                                in1=accA[:, b, 0:rhsN],
                                                op=Alu.add)
                        group_done(b)
                    cur["psum"] = None

                for (hf, t0, t1) in ((0, 0, S_A), (1, S_A, S)):
                    tblh = t_full[hf][:]
                    for (c0, cn) in _chunks(t0, t1):
                        ni = cn * TILE
                        isb = pisb.tile([128, CH * 8], dt.int16, tag="isrc")
                        nc.sync.dma_start(
                            out=isb[:, 0:cn * 8],
                            in_=isrc_in[:, c0 * 8:c0 * 8 + cn * 8])
                        ohb = poh.tile([128, CH * TILE], dt.bfloat16,
                                       tag="oh")
                        nc.vector.tensor_tensor(
                            out=ohb[:, 0:cn * TILE].rearrange(
                                "p (c j) -> p c j", j=TILE),
                            in0=dcd[:, c0:c0 + cn].rearrange(
                                "p (c o) -> p c o", o=1).to_broadcast(
                                    [128, cn, TILE]),
                            in1=iot[:].rearrange("p (o j) -> p o j",
                                                 o=1).to_broadcast(
                                                     [128, cn, TILE]),
                            op=Alu.is_equal)
                        ohTb = pohT.tile([128, CH * TILE], dt.bfloat16,
                                         tag="ohT")
                        nc.sync.dma_start(
                            out=ohTb[:, 0:cn * TILE],
                            in_=ohT_in[:, c0 * TILE:(c0 + cn) * TILE])
                        gath = pgath.tile([128, CH, TCOL], dt.int16,
                                          tag="gath")
                        nc.gpsimd.dma_gather(
                            out_ap=gath[:, 0:cn, :], in_ap=tblh,
                            idxs_ap=isb[:, 0:cn * 8], num_idxs=ni,
                            num_idxs_reg=ni, elem_size=TCOL,
                            single_packet=False,
                            queue_num=chunk_no[0] % 4)
                        chunk_no[0] += 1
                        gbf = gath[:, 0:cn, :].bitcast(dt.bfloat16)
                        # er[dst] per slot: per-tile ohT.T @ er_block
                        per = psB.tile([128, CH * 4], dt.float32, tag="er",
                                       name="erps", space="PSUM")
                        for t in range(cn):
                            nc.tensor.matmul(
                                per[:, t * H:(t + 1) * H],
                                lhsT=ohTb[:, t * TILE:(t + 1) * TILE],
                                rhs=er_sb[:, tb[c0 + t], 0:H],
                                start=True, stop=True)
                        est = pest.tile([128, CH, 4], dt.float32, tag="est")
                        nc.vector.tensor_tensor(
                            out=est[:, 0:cn, 0:H],
                            in0=gbf[:, :, ELC:ELC + H],
                            in1=per[:, 0:cn * H].rearrange(
                                "p (c h) -> p c h", h=H),
                            op=Alu.add)
                        nc.vector.scalar_tensor_tensor(
                            out=est[:, 0:cn, 0:H], in0=est[:, 0:cn, 0:H],
                            scalar=NEG, in1=est[:, 0:cn, 0:H],
                            op0=Alu.mult, op1=Alu.max)
                        wt = pwt.tile([128, CH, rhsN], dt.bfloat16, tag="wt")
                        nc.scalar.activation(wt[:, 0:cn, HD:HD + H],
                                             est[:, 0:cn, 0:H], Act.Exp)
                        if quant:
                            # exs = ex * scale (dequant folded in)
                            nc.vector.tensor_tensor(
                                out=est[:, 0:cn, 0:H],
                                in0=wt[:, 0:cn, HD:HD + H],
                                in1=gbf[:, :, SCC:SCC + H],
                                op=Alu.mult)
                            fsrc = gath[:, 0:cn, :].bitcast(dt.int8)[
                                :, :, 0:128].rearrange(
                                    "p c (h d) -> p c h d", h=H)
                            mul_in1 = est[:, 0:cn, 0:H].rearrange(
                                "p c (h o) -> p c h o", h=H).to_broadcast(
                                    [128, cn, H, D])
                        else:
                            fsrc = gbf[:, :, 0:HD].rearrange(
                                "p c (h d) -> p c h d", h=H)
                            mul_in1 = wt[:, 0:cn, HD:HD + H].rearrange(
                                "p c (h o) -> p c h o", h=H).to_broadcast(
                                    [128, cn, H, D])
                        nc.vector.tensor_tensor(
                            out=wt[:, 0:cn, 0:HD].rearrange(
                                "p c (h d) -> p c h d", h=H),
                            in0=fsrc, in1=mul_in1, op=Alu.mult)
                        for t in range(cn):
                            g = c0 + t
                            if tst[g]:
                                finish_block()
                                cur["psum"] = psA.tile(
                                    [128, rhsN], dt.float32, tag="agg",
                                    name="aggp", space="PSUM")
                                cur["b"], cur["half"] = tb[g], hf
                            nc.tensor.matmul(
                                cur["psum"][:],
                                lhsT=ohb[:, t * TILE:(t + 1) * TILE],
                                rhs=wt[:, t, 0:rhsN],
                                start=tst[g], stop=tsp[g])
                    finish_block()
                flush_pending()

            # ---- layer 1 projection from streamed xT blocks ----
            def xlhs(b, k):
                xb = xblkp.tile([128, 2, 128], dt.bfloat16, tag="xb",
                                name=f"xb{b}")
                if k == 0:
                    nc.sync.dma_start(
                        out=xb[:],
                        in_=xT_in[:].rearrange("p (k c) -> p k c",
                                               k=2)[:, :, b * BLK:(b + 1) * BLK])
                    xlhs.cache[b] = xb
                return xlhs.cache[b][:, k, :]
            xlhs.cache = {}

            for i, (b0, b1) in enumerate(GROUPS):
                proj_stage(0, b0, b1, xlhs, PBUFS[i % 2])
                quant_flush(0, b0, b1, PBUFS[i % 2])
                if b1 == PSB:
                    store_part(0)
            store_part(1)
            edge_phase(0)      # overlaps layer-2 proj + collectives
            edge_phase(1)      # overlaps layer-3 proj + collectives
            edge_phase(2)      # writes output per group
    nc.finalize()
    return nc


def kernel(**inputs):
    x = np.asarray(inputs["x"], f32)
    src = np.asarray(inputs["src"]).astype(np.int64)
    dst = np.asarray(inputs["dst"]).astype(np.int64)

    meta, cores = _structure(src, dst)

    def wext(W, al, ar):
        W = np.asarray(W, f32)
        al = np.asarray(al, f32)
        ar = np.asarray(ar, f32)
        Hh, Dd = al.shape
        Wl = np.stack([W[:, h * Dd:(h + 1) * Dd] @ al[h] for h in range(Hh)],
                      1)
        Wr = np.stack([W[:, h * Dd:(h + 1) * Dd] @ ar[h] for h in range(Hh)],
                      1)
        return np.concatenate([W, Wl, Wr], axis=1)

    wx = [wext(inputs["W1"], inputs["al1"], inputs["ar1"]),
          wext(inputs["W2"], inputs["al2"], inputs["ar2"]),
          wext(inputs["W3"], inputs["al3"], inputs["ar3"])]
    w_arrs = []
    for i, cfg in enumerate(LAYERS):
        kt, nw = cfg["kt"], cfg["HD"] + 2 * cfg["H"]
        a = np.zeros((128, kt, nw), bf16)
        for k in range(kt):
            a[:, k, :] = wx[i][k * 128:(k + 1) * 128, :].astype(bf16)
        w_arrs.append(a.reshape(128, kt * nw))
    b_arrs = [np.tile(np.asarray(inputs[f"b{i+1}"], f32).reshape(1, -1),
                      (128, 1)) for i in range(3)]

    nc = _build_program(meta)

    iota_arr = np.tile(np.arange(128, dtype=bf16).reshape(1, 128), (128, 1))
    in_maps = []
    for k in range(NCORES):
        lo = k * NLOC
        xT = np.zeros((128, 2, NLOC_PAD), bf16)
        xs = x[lo:lo + NLOC].astype(bf16)
        for kk in range(2):
            xT[:, kk, 0:NLOC] = xs[:, kk * 128:(kk + 1) * 128].T
        in_maps.append({
            "xT": xT.reshape(128, 2 * NLOC_PAD),
            "W1": w_arrs[0], "W2": w_arrs[1], "W3": w_arrs[2],
            "b1": b_arrs[0], "b2": b_arrs[1], "b3": b_arrs[2],
            "idx_src": cores[k]["idx_src"],
            "dcode": cores[k]["dcode"],
            "iota": iota_arr,
            "ohT": cores[k]["ohT"],
        })

    if os.environ.get("KGAT_SIM"):
        from concourse import bass2jax
        results = bass2jax.run_bass_via_pjrt(nc, in_maps, n_cores=NCORES)
        res = SimpleNamespace(results=results, exec_time_ns=None,
                              instructions_and_trace=None)
    else:
        trace = bool(os.environ.get("KGAT_TRACE"))
        res = run_bass_kernel_spmd(nc, in_maps, core_ids=list(range(NCORES)),
                                   trace=trace)
    global LAST_RESULTS
    LAST_RESULTS = res
    out = np.concatenate([res.results[k]["out"][:NLOC]
                          for k in range(NCORES)], axis=0)
    return out.astype(f32)


LAST_RESULTS = None


# revision 12
# speedup vs baseline: 1.3174x; 1.0355x over previous
"""3-layer GAT on 8 Trainium2 NeuronCores (Bass/Tile).

Strategy (dst-node graph partition):
  - Each core owns a contiguous slice of N/8 dst nodes and all edges into
    them. Per layer, nodes are projected data-parallel with an extended
    weight [W | W@al | W@ar]; per-node table rows are packed into 256-BYTE
    elements (the dma_gather sweet spot: ~3 ns/edge vs ~10 ns/edge for
    512B): layers 1-2 store feat as per-(node,head)-scaled int8 plus bf16
    el and bf16 scale; layer 3 stores bf16 feat + el directly. AllGather
    replicates the table; per-edge rows are fetched with dma_gather from
    two half-tables (keeps indices within int16).
  - Attention uses exp without max-subtraction (shift-invariant softmax,
    |e| small); per-128-edge-tile one-hot matmuls accumulate ex-weighted
    feature sums and softmax denominators into per-block PSUM. The int8
    feat is dequantized for free by the ex*scale multiply on Vector.
  - Epilogue (softmax divide, bias, ELU) runs in bulk per ~10-block group,
    and the next layer's projection of each finished group overlaps the
    remaining edge phase; only the AllGather sits between layers.
"""
import os
import sys
from types import SimpleNamespace

import numpy as np
import ml_dtypes

try:
    from concourse import bass, mybir, tile, bacc  # noqa: F401
except ImportError:  # pragma: no cover
    sys.path.insert(0, "/opt/trn_rl_repo")
    from concourse import bass, mybir, tile, bacc  # noqa: F401
from concourse.bass_utils import run_bass_kernel_spmd

bf16 = ml_dtypes.bfloat16
f32 = np.float32

N = int(os.environ.get("KGAT_N", "50000"))
E = int(os.environ.get("KGAT_E", "800000"))
NEG = 0.2
NCORES = 8
NLOC = N // NCORES
BLK = 128
NBLK = (NLOC + BLK - 1) // BLK
NLOC_PAD = NBLK * BLK
TILE = 128
CH = 32                      # tiles per gather chunk
TCOL = 128                   # int16 cols per table row (256 bytes)
RND = 1.5 * 2.0 ** 23        # f32 round-to-nearest-int magic constant

# src-side split: table part 0 = local blocks [0,PSB), part 1 = [PSB,NBLK);
# each part AllGathers into its own tensor (rows fit int16) and part 0's
# collective fires mid-edge-phase, overlapped with remaining gathers.
PSB = (NBLK + 1) // 2
P_ROWS = [PSB * BLK, (NBLK - PSB) * BLK]
P_SPLIT = PSB * BLK
assert NCORES * max(P_ROWS) < 32768

NGRP_PER_PART = 3
GROUPS = [(int(g[0]), int(g[-1]) + 1)
          for rng in (np.arange(PSB), np.arange(PSB, NBLK))
          for g in np.array_split(rng, min(NGRP_PER_PART, len(rng)))
          if len(g)]
GMAX = max(b1 - b0 for b0, b1 in GROUPS)

# layer configs; table row (int16 cols): quant: [feat i8 x128 (cols 0:64) |
# el bf16 xH (64:64+H) | scale bf16 xH (68:68+H)]; else bf16 [feat | el].
LAYERS = [
    dict(kt=2, H=4, D=32, HD=128, quant=True),
    dict(kt=1, H=4, D=32, HD=128, quant=True),
    dict(kt=1, H=1, D=64, HD=64, quant=False),
]
ELC = 64     # bf16 col where el starts (byte 128)
SCC = 68     # bf16 col where scale starts (quant layers)


def _wrap_idx(vals):
    n = len(vals)
    assert n % 16 == 0
    arr = np.asarray(vals, np.int16).reshape(-1, 16).T
    return np.tile(arr, (8, 1))


def _structure(src, dst):
    """Shared tile schedule + per-core index/one-hot arrays."""
    counts = np.zeros((NCORES, NBLK, 2), np.int64)
    per_core = []
    for k in range(NCORES):
        lo = k * NLOC
        m = (dst >= lo) & (dst < lo + NLOC)
        eidx = np.nonzero(m)[0]
        d_loc = dst[eidx] - lo
        half = ((src[eidx] % NLOC) >= P_SPLIT).astype(np.int64)
        blk = d_loc // BLK
        order = np.lexsort((d_loc, blk, half))
        eidx, d_loc, half, blk = (a[order] for a in (eidx, d_loc, half, blk))
        per_core.append((eidx, d_loc, half, blk))
        np.add.at(counts[k], (blk, half), 1)
    T = np.maximum(np.ceil(counts / TILE).astype(np.int64).max(axis=0), 1)

    tile_block, tile_start, tile_stop = [], [], []
    for h in range(2):
        for b in range(NBLK):
            for t in range(T[b, h]):
                tile_block.append(b)
                tile_start.append(t == 0)
                tile_stop.append(t == T[b, h] - 1)
    S = len(tile_block)
    S_A = int(T[:, 0].sum())

    cores = []
    for k in range(NCORES):
        eidx, d_loc, half, blk = per_core[k]
        src_rows = np.zeros(S * TILE, np.int64)
        oh = np.zeros((128, S * TILE), bf16)
        dcode = np.full((128, S), -1.0, bf16)
        pos = 0
        for h in range(2):
            for b in range(NBLK):
                sel = np.nonzero((blk == b) & (half == h))[0]
                ns = len(sel)
                sl = slice(pos, pos + ns)
                s_glob = src[eidx[sel]]
                loc = s_glob % NLOC
                r = ((s_glob // NLOC) * P_ROWS[h] + loc
                     - (P_SPLIT if h else 0))
                src_rows[sl] = r
                slots = pos + np.arange(ns)
                dc = d_loc[sel] - b * BLK
                oh[slots % 128, (slots // 128) * 128 + dc] = 1.0
                dcode[slots % 128, slots // 128] = dc
                pos += T[b, h] * TILE
        assert src_rows.max() < 32768 and src_rows.min() >= 0
        ohT = np.ascontiguousarray(
            oh.reshape(128, S, TILE).transpose(2, 1, 0)).reshape(
                128, S * TILE).astype(np.int8)
        cores.append(dict(idx_src=_wrap_idx(src_rows), dcode=dcode, ohT=ohT))
    meta = dict(T=T, S=S, S_A=S_A, tile_block=tile_block,
                tile_start=tile_start, tile_stop=tile_stop)
    return meta, cores


def _chunks(t0, t1):
    out = []
    t = t0
    while t < t1:
        c = min(CH, t1 - t)
        out.append((t, c))
        t += c
    return out


def _build_program(meta):
    from concourse.masks import make_identity
    dt = mybir.dt
    Alu = mybir.AluOpType
    Act = mybir.ActivationFunctionType
    S, S_A = meta["S"], meta["S_A"]
    tb, tst, tsp = meta["tile_block"], meta["tile_start"], meta["tile_stop"]

    nc = bacc.Bacc("TRN2", target_bir_lowering=False, debug=False,
                   num_devices=NCORES, num_swdge_queues=4)
    xT_in = nc.dram_tensor("xT", [128, 2 * NLOC_PAD], dt.bfloat16,
                           kind="ExternalInput")
    w_in = [nc.dram_tensor(f"W{i+1}", [128, LAYERS[i]["kt"] * (
        LAYERS[i]["HD"] + 2 * LAYERS[i]["H"])], dt.bfloat16,
        kind="ExternalInput") for i in range(3)]
    b_in = [nc.dram_tensor(f"b{i+1}", [128, LAYERS[i]["HD"]], dt.float32,
                           kind="ExternalInput") for i in range(3)]
    isrc_in = nc.dram_tensor("idx_src", [128, S * 8], dt.int16,
                             kind="ExternalInput")
    dcode_in = nc.dram_tensor("dcode", [128, S], dt.bfloat16,
                              kind="ExternalInput")
    iota_in = nc.dram_tensor("iota", [128, 128], dt.bfloat16,
                             kind="ExternalInput")
    ohT_in = nc.dram_tensor("ohT", [128, S * TILE], dt.int8,
                            kind="ExternalInput")
    out_ext = nc.dram_tensor("out", [NLOC_PAD, 64], dt.float32,
                             kind="ExternalOutput")

    with tile.TileContext(nc) as tc:
        with (
            tc.tile_pool(name="const", bufs=1) as constp,
            tc.tile_pool(name="xblkp", bufs=3) as xblkp,
            tc.tile_pool(name="stage", bufs=1) as stagep,
            tc.tile_pool(name="epi", bufs=1) as epip,
            tc.tile_pool(name="pgath", bufs=6) as pgath,
            tc.tile_pool(name="pwt", bufs=3) as pwt,
            tc.tile_pool(name="poh", bufs=3) as poh,
            tc.tile_pool(name="pohT", bufs=3) as pohT,
            tc.tile_pool(name="pohT8", bufs=3) as pohT8,
            tc.tile_pool(name="pisb", bufs=4) as pisb,
            tc.tile_pool(name="pest", bufs=2) as pest,
            tc.tile_pool(name="psA", bufs=2, space="PSUM") as psA,
            tc.tile_pool(name="psB", bufs=2, space="PSUM") as psB,
            tc.tile_pool(name="dram", bufs=1, space="DRAM") as dram,
        ):
            ident = constp.tile([128, 128], dt.bfloat16, tag="ident")
            make_identity(nc, ident[:])
            w_sb, b_sb = [], []
            for i, cfg in enumerate(LAYERS):
                nw = cfg["HD"] + 2 * cfg["H"]
                w = constp.tile([128, cfg["kt"], nw], dt.bfloat16,
                                tag=f"w{i}")
                nc.sync.dma_start(out=w[:], in_=w_in[i][:].rearrange(
                    "p (k c) -> p k c", k=cfg["kt"]))
                w_sb.append(w)
                bb = constp.tile([128, cfg["HD"]], dt.float32, tag=f"b{i}")
                nc.sync.dma_start(out=bb[:], in_=b_in[i][:])
                b_sb.append(bb)

            tbl_sb = stagep.tile([128, NBLK, TCOL], dt.int16, tag="tblsb")
            nc.vector.memset(tbl_sb[:], 0.0)
            tbl_bf = tbl_sb[:].bitcast(dt.bfloat16)
            tbl_i8 = tbl_sb[:].bitcast(dt.int8)
            er_sb = stagep.tile([128, NBLK, 4], dt.bfloat16, tag="ers")
            accA = stagep.tile([128, NBLK, 132], dt.float32, tag="accA")
            fstage0 = stagep.tile([128, GMAX, 128], dt.float32, tag="fst0")
            elerst0 = stagep.tile([128, GMAX, 8], dt.float32, tag="elerst0")
            fstage1 = stagep.tile([128, GMAX, 128], dt.float32, tag="fst1")
            elerst1 = stagep.tile([128, GMAX, 8], dt.float32, tag="elerst1")
            PBUFS = [(fstage0, elerst0), (fstage1, elerst1)]

            t_loc = [dram.tile([P_ROWS[0], TCOL], dt.int16, tag="tloc0",
                               name="tloc0"),
                     dram.tile([P_ROWS[1], TCOL], dt.int16, tag="tloc1",
                               name="tloc1")]
            t_full = [dram.tile([NCORES * P_ROWS[0], TCOL], dt.int16,
                                tag="tfull0", name="tfull0"),
                      dram.tile([NCORES * P_ROWS[1], TCOL], dt.int16,
                                tag="tfull1", name="tfull1")]
            dcd = constp.tile([128, S], dt.bfloat16, tag="dcd")
            nc.sync.dma_start(out=dcd[:], in_=dcode_in[:])
            iot = constp.tile([128, 128], dt.bfloat16, tag="iot")
            nc.sync.dma_start(out=iot[:], in_=iota_in[:])

            def proj_stage(li, b0, b1, lhs_of, pp_buf):
                """Projection matmuls + staging for blocks [b0,b1)."""
                cfg = LAYERS[li]
                H, HD, kt, quant = cfg["H"], cfg["HD"], cfg["kt"], cfg["quant"]
                fst, elst = pp_buf
                for b in range(b0, b1):
                    pp = psB.tile([128, HD + 2 * H], dt.float32, tag="proj",
                                  name="projpp", space="PSUM")
                    for k in range(kt):
                        nc.tensor.matmul(pp[:], lhsT=lhs_of(b, k),
                                         rhs=w_sb[li][:, k, :],
                                         start=(k == 0), stop=(k == kt - 1))
                    if quant:
                        nc.scalar.activation(fst[:, b - b0, 0:HD],
                                             pp[:, 0:HD], Act.Copy)
                        nc.vector.tensor_copy(out=elst[:, b - b0, 0:2 * H],
                                              in_=pp[:, HD:HD + 2 * H])
                    else:
                        nc.scalar.activation(tbl_bf[:, b, 0:HD + H],
                                             pp[:, 0:HD + H], Act.Copy)
                        nc.vector.tensor_copy(out=er_sb[:, b, 0:H],
                                              in_=pp[:, HD + H:HD + 2 * H])

            def quant_flush(li, b0, b1, pp_buf):
                """Quantize staged blocks into the int8 table (quant layers)."""
                cfg = LAYERS[li]
                H = cfg["H"]
                G = b1 - b0
                fst, elst = pp_buf
                fv = fst[:, 0:G, :].rearrange("p g (h d) -> p g h d", h=H)
                mx = epip.tile([128, GMAX, 4], dt.float32, tag="mx",
                               name="mxt")
                nc.vector.tensor_reduce(out=mx[:, 0:G, :], in_=fv,
                                        axis=mybir.AxisListType.X,
                                        op=Alu.max,
                                        apply_absolute_value=True)
                nc.vector.tensor_scalar_max(out=mx[:, 0:G, :],
                                            in0=mx[:, 0:G, :],
                                            scalar1=1e-20)
                # scale (bf16, stored in table) then rs = 1/scale
                nc.scalar.activation(tbl_bf[:, b0:b1, SCC:SCC + H],
                                     mx[:, 0:G, :], Act.Copy,
                                     scale=1.0 / 127.0)
                rs = epip.tile([128, GMAX, 4], dt.float32, tag="rs",
                               name="rst")
                nc.vector.reciprocal(out=rs[:, 0:G, :],
                                     in_=tbl_bf[:, b0:b1, SCC:SCC + H])
                # qint = round(feat * rs): mult, then +RND -RND trick
                nc.vector.tensor_tensor(
                    out=fv, in0=fv,
                    in1=rs[:, 0:G, :].rearrange("p g (h o) -> p g h o", h=H)
                    .to_broadcast([128, G, H, cfg["D"]]),
                    op=Alu.mult)
                nc.vector.tensor_scalar_add(out=fv, in0=fv, scalar1=RND)
                nc.vector.tensor_scalar_add(out=fv, in0=fv, scalar1=-RND)
                nc.vector.tensor_copy(
                    out=tbl_i8[:, b0:b1, 0:128].rearrange(
                        "p g (h d) -> p g h d", h=H), in_=fv)
                nc.vector.tensor_copy(out=tbl_bf[:, b0:b1, ELC:ELC + H],
                                      in_=elst[:, 0:G, 0:H])
                nc.vector.tensor_copy(out=er_sb[:, b0:b1, 0:H],
                                      in_=elst[:, 0:G, H:2 * H])

            def store_part(part):
                b0, b1 = (0, PSB) if part == 0 else (PSB, NBLK)
                nc.sync.dma_start(
                    out=t_loc[part][:].rearrange("(b p) c -> p b c", p=128),
                    in_=tbl_sb[:, b0:b1, :])
                nc.gpsimd.collective_compute(
                    "AllGather", mybir.AluOpType.bypass,
                    replica_groups=[list(range(NCORES))],
                    ins=[t_loc[part][:].opt()],
                    outs=[t_full[part][:].opt()])

            def epilogue_range(li, b0, b1):
                """Softmax-divide + bias (+ELU+transpose or output DMA)."""
                cfg = LAYERS[li]
                H, HD = cfg["H"], cfg["HD"]
                G = b1 - b0
                last = li == 2
                dr = epip.tile([128, GMAX, 4], dt.float32, tag="dr",
                               name="drt")
                nc.vector.tensor_scalar_add(out=dr[:, 0:G, 0:H],
                                            in0=accA[:, b0:b1, HD:HD + H],
                                            scalar1=1e-9)
                nc.vector.reciprocal(out=dr[:, 0:G, 0:H],
                                     in_=dr[:, 0:G, 0:H])
                qt = epip.tile([128, GMAX, 128], dt.float32, tag="qt",
                               name="qtt")
                nc.vector.tensor_tensor(
                    out=qt[:, 0:G, 0:HD].rearrange("p g (h d) -> p g h d",
                                                   h=H),
                    in0=accA[:, b0:b1, 0:HD].rearrange("p g (h d) -> p g h d",
                                                       h=H),
                    in1=dr[:, 0:G, 0:H].rearrange("p g (h o) -> p g h o",
                                                  h=H)
                    .to_broadcast([128, G, H, cfg["D"]]),
                    op=Alu.mult)
                nc.vector.tensor_tensor(
                    out=qt[:, 0:G, 0:HD], in0=qt[:, 0:G, 0:HD],
                    in1=b_sb[li][:].rearrange("p (o c) -> p o c", o=1)
                    .to_broadcast([128, G, HD]),
                    op=Alu.add)
                if last:
                    nc.sync.dma_start(
                        out=out_ext[:].rearrange("(b p) c -> p b c",
                                                 p=128)[:, b0:b1, :],
                        in_=qt[:, 0:G, 0:64])
                    return None
                # elu: relu(q) + exp(min(q,0)) - 1
                m = epip.tile([128, GMAX, 128], dt.float32, tag="m",
                              name="mt")
                nc.vector.tensor_scalar_min(out=m[:, 0:G, 0:HD],
                                            in0=qt[:, 0:G, 0:HD], scalar1=0.0)
                nc.scalar.activation(m[:, 0:G, 0:HD], m[:, 0:G, 0:HD],
                                     Act.Exp)
                hb = epip.tile([128, GMAX, 128], dt.float32, tag="hb",
                               name="hbt")
                nc.vector.scalar_tensor_tensor(
                    out=hb[:, 0:G, 0:HD], in0=qt[:, 0:G, 0:HD], scalar=0.0,
                    in1=m[:, 0:G, 0:HD], op0=Alu.max, op1=Alu.add)
                hbb = epip.tile([128, GMAX, 128], dt.bfloat16, tag="hbb",
                                name="hbbt")
                nc.vector.tensor_scalar_add(out=hbb[:, 0:G, 0:HD],
                                            in0=hb[:, 0:G, 0:HD],
                                            scalar1=-1.0)
                hgrp = epip.tile([128, GMAX * 128], dt.bfloat16, tag="hgrp",
                                 name="hgrpt")
                for b in range(b0, b1):
                    tp = psB.tile([128, 128], dt.bfloat16, tag="tp",
                                  name="tpt", space="PSUM")
                    nc.tensor.transpose(tp[:], hbb[:, b - b0, :], ident[:])
                    nc.scalar.activation(
                        hgrp[:, (b - b0) * 128:(b - b0 + 1) * 128],
                        tp[:], Act.Copy)
                return hgrp

            def edge_phase(li):
                cfg = LAYERS[li]
                H, D, HD, quant = cfg["H"], cfg["D"], cfg["HD"], cfg["quant"]
                rhsN = HD + H
                last = li == 2
                cur = {"psum": None, "b": None, "half": None}
                chunk_no = [0]
                gi = [0]

                pending = []

                def flush_pending():
                    if not pending:
                        return
                    p = pending.pop()
                    if LAYERS[p[0]]["quant"]:
                        quant_flush(*p[:3], p[3])
                    if p[2] == PSB:
                        store_part(0)
                    elif p[2] == NBLK:
                        store_part(1)

                def group_done(b):
                    if gi[0] < len(GROUPS) and b == GROUPS[gi[0]][1] - 1:
                        b0, b1 = GROUPS[gi[0]]
                        flush_pending()
                        hgrp = epilogue_range(li, b0, b1)
                        if not last:
                            nli = li + 1
                            def lhs_of(bb, k, hgrp=hgrp, b0=b0):
                                return hgrp[:, (bb - b0) * 128:
                                            (bb - b0 + 1) * 128]
                            proj_stage(nli, b0, b1, lhs_of,
                                       PBUFS[gi[0] % 2])
                            pending.append((nli, b0, b1, PBUFS[gi[0] % 2]))
                        gi[0] += 1

                def finish_block():
                    ps, b, half = cur["psum"], cur["b"], cur["half"]
                    if ps is None:
                        return
                    if half == 0:
                        nc.scalar.activation(accA[:, b, 0:rhsN], ps[:],
                                             Act.Copy)
                    else:
                        nc.vector.tensor_tensor(out=accA[:, b, 0:rhsN],
                                                in0=ps[:],
                                                in1=accA[:, b, 0:rhsN],
                                                op=Alu.add)
                        group_done(b)
                    cur["psum"] = None

                for (hf, t0, t1) in ((0, 0, S_A), (1, S_A, S)):
                    tblh = t_full[hf][:]
                    for (c0, cn) in _chunks(t0, t1):
                        ni = cn * TILE
                        isb = pisb.tile([128, CH * 8], dt.int16, tag="isrc")
                        nc.sync.dma_start(
                            out=isb[:, 0:cn * 8],
                            in_=isrc_in[:, c0 * 8:c0 * 8 + cn * 8])
                        ohb = poh.tile([128, CH * TILE], dt.bfloat16,
                                       tag="oh")
                        nc.vector.tensor_tensor(
                            out=ohb[:, 0:cn * TILE].rearrange(
                                "p (c j) -> p c j", j=TILE),
                            in0=dcd[:, c0:c0 + cn].rearrange(
                                "p (c o) -> p c o", o=1).to_broadcast(
                                    [128, cn, TILE]),
                            in1=iot[:].rearrange("p (o j) -> p o j",
                                                 o=1).to_broadcast(
                                                     [128, cn, TILE]),
                            op=Alu.is_equal)
                        ohT8 = pohT8.tile([128, CH * TILE], dt.int8,
                                          tag="ohT8")
                        nc.sync.dma_start(
                            out=ohT8[:, 0:cn * TILE],
                            in_=ohT_in[:, c0 * TILE:(c0 + cn) * TILE])
                        ohTb = pohT.tile([128, CH * TILE], dt.bfloat16,
                                         tag="ohT")
                        nc.scalar.activation(ohTb[:, 0:cn * TILE],
                                             ohT8[:, 0:cn * TILE], Act.Copy)
                        gath = pgath.tile([128, CH, TCOL], dt.int16,
                                          tag="gath")
                        nc.gpsimd.dma_gather(
                            out_ap=gath[:, 0:cn, :], in_ap=tblh,
                            idxs_ap=isb[:, 0:cn * 8], num_idxs=ni,
                            num_idxs_reg=ni, elem_size=TCOL,
                            single_packet=False,
                            queue_num=chunk_no[0] % 4)
                        chunk_no[0] += 1
                        gbf = gath[:, 0:cn, :].bitcast(dt.bfloat16)
                        # er[dst] per slot: per-tile ohT.T @ er_block
                        per = psB.tile([128, CH * 4], dt.float32, tag="er",
                                       name="erps", space="PSUM")
                        for t in range(cn):
                            nc.tensor.matmul(
                                per[:, t * H:(t + 1) * H],
                                lhsT=ohTb[:, t * TILE:(t + 1) * TILE],
                                rhs=er_sb[:, tb[c0 + t], 0:H],
                                start=True, stop=True)
                        est = pest.tile([128, CH, 4], dt.float32, tag="est")
                        nc.vector.tensor_tensor(
                            out=est[:, 0:cn, 0:H],
                            in0=gbf[:, :, ELC:ELC + H],
                            in1=per[:, 0:cn * H].rearrange(
                                "p (c h) -> p c h", h=H),
                            op=Alu.add)
                        nc.vector.scalar_tensor_tensor(
                            out=est[:, 0:cn, 0:H], in0=est[:, 0:cn, 0:H],
                            scalar=NEG, in1=est[:, 0:cn, 0:H],
                            op0=Alu.mult, op1=Alu.max)
                        wt = pwt.tile([128, CH, rhsN], dt.bfloat16, tag="wt")
                        nc.scalar.activation(wt[:, 0:cn, HD:HD + H],
                                             est[:, 0:cn, 0:H], Act.Exp)
                        if quant:
                            # exs = ex * scale (dequant folded in)
                            nc.vector.tensor_tensor(
                                out=est[:, 0:cn, 0:H],
                                in0=wt[:, 0:cn, HD:HD + H],
                                in1=gbf[:, :, SCC:SCC + H],
                                op=Alu.mult)
                            fsrc = gath[:, 0:cn, :].bitcast(dt.int8)[
                                :, :, 0:128].rearrange(
                                    "p c (h d) -> p c h d", h=H)
                            mul_in1 = est[:, 0:cn, 0:H].rearrange(
                                "p c (h o) -> p c h o", h=H).to_broadcast(
                                    [128, cn, H, D])
                        else:
                            fsrc = gbf[:, :, 0:HD].rearrange(
                                "p c (h d) -> p c h d", h=H)
                            mul_in1 = wt[:, 0:cn, HD:HD + H].rearrange(
                                "p c (h o) -> p c h o", h=H).to_broadcast(
                                    [128, cn, H, D])
                        nc.vector.tensor_tensor(
                            out=wt[:, 0:cn, 0:HD].rearrange(
                                "p c (h d) -> p c h d", h=H),
                            in0=fsrc, in1=mul_in1, op=Alu.mult)
                        for t in range(cn):
                            g = c0 + t
                            if tst[g]:
                                finish_block()
                                cur["psum"] = psA.tile(
                                    [128, rhsN], dt.float32, tag="agg",
                                    name="aggp", space="PSUM")
                                cur["b"], cur["half"] = tb[g], hf
                            nc.tensor.matmul(
                                cur["psum"][:],
                                lhsT=ohb[:, t * TILE:(t + 1) * TILE],
                                rhs=wt[:, t, 0:rhsN],
                                start=tst[g], stop=tsp[g])
                    finish_block()
                flush_pending()

            # ---- layer 1 projection from streamed xT blocks ----
            def xlhs(b, k):
                xb = xblkp.tile([128, 2, 128], dt.bfloat16, tag="xb",
                                name=f"xb{b}")
                if k == 0:
                    nc.sync.dma_start(
                        out=xb[:],
                        in_=xT_in[:].rearrange("p (k c) -> p k c",
                                               k=2)[:, :, b * BLK:(b + 1) * BLK])
                    xlhs.cache[b] = xb
                return xlhs.cache[b][:, k, :]
            xlhs.cache = {}

            for i, (b0, b1) in enumerate(GROUPS):
                proj_stage(0, b0, b1, xlhs, PBUFS[i % 2])
                quant_flush(0, b0, b1, PBUFS[i % 2])
                if b1 == PSB:
                    store_part(0)
            store_part(1)
            edge_phase(0)      # overlaps layer-2 proj + collectives
            edge_phase(1)      # overlaps layer-3 proj + collectives
            edge_phase(2)      # writes output per group
    nc.finalize()
    return nc


def kernel(**inputs):
    x = np.asarray(inputs["x"], f32)
    src = np.asarray(inputs["src"]).astype(np.int64)
    dst = np.asarray(inputs["dst"]).astype(np.int64)

    meta, cores = _structure(src, dst)

    def wext(W, al, ar):
        W = np.asarray(W, f32)
        al = np.asarray(al, f32)
        ar = np.asarray(ar, f32)
        Hh, Dd = al.shape
        Wl = np.stack([W[:, h * Dd:(h + 1) * Dd] @ al[h] for h in range(Hh)],
                      1)
        Wr = np.stack([W[:, h * Dd:(h + 1) * Dd] @ ar[h] for h in range(Hh)],
                      1)
        return np.concatenate([W, Wl, Wr], axis=1)

    wx = [wext(inputs["W1"], inputs["al1"], inputs["ar1"]),
          wext(inputs["W2"], inputs["al2"], inputs["ar2"]),
          wext(inputs["W3"], inputs["al3"], inputs["ar3"])]
    w_arrs = []
    for i, cfg in enumerate(LAYERS):
        kt, nw = cfg["kt"], cfg["HD"] + 2 * cfg["H"]
        a = np.zeros((128, kt, nw), bf16)
        for k in range(kt):
            a[:, k, :] = wx[i][k * 128:(k + 1) * 128, :].astype(bf16)
        w_arrs.append(a.reshape(128, kt * nw))
    b_arrs = [np.tile(np.asarray(inputs[f"b{i+1}"], f32).reshape(1, -1),
                      (128, 1)) for i in range(3)]

    nc = _build_program(meta)

    iota_arr = np.tile(np.arange(128, dtype=bf16).reshape(1, 128), (128, 1))
    in_maps = []
    for k in range(NCORES):
        lo = k * NLOC
        xT = np.zeros((128, 2, NLOC_PAD), bf16)
        xs = x[lo:lo + NLOC].astype(bf16)
        for kk in range(2):
            xT[:, kk, 0:NLOC] = xs[:, kk * 128:(kk + 1) * 128].T
        in_maps.append({
            "xT": xT.reshape(128, 2 * NLOC_PAD),
            "W1": w_arrs[0], "W2": w_arrs[1], "W3": w_arrs[2],
            "b1": b_arrs[0], "b2": b_arrs[1], "b3": b_arrs[2],
            "idx_src": cores[k]["idx_src"],
            "dcode": cores[k]["dcode"],
            "iota": iota_arr,
            "ohT": cores[k]["ohT"],
        })

    if os.environ.get("KGAT_SIM"):
        from concourse import bass2jax
        results = bass2jax.run_bass_via_pjrt(nc, in_maps, n_cores=NCORES)
        res = SimpleNamespace(results=results, exec_time_ns=None,
                              instructions_and_trace=None)
    else:
        trace = bool(os.environ.get("KGAT_TRACE"))
        res = run_bass_kernel_spmd(nc, in_maps, core_ids=list(range(NCORES)),
                                   trace=trace)
    global LAST_RESULTS
    LAST_RESULTS = res
    out = np.concatenate([res.results[k]["out"][:NLOC]
                          for k in range(NCORES)], axis=0)
    return out.astype(f32)


LAST_RESULTS = None
